# revision 4
# baseline (speedup 1.0000x reference)
"""Causal self-attention (B=4, T=2048, D=2048, H=16, HD=128) on 8 Trainium2
NeuronCores.

Sharding: Megatron-style tensor parallel over heads for QKV projection +
attention (2 heads per core), then an on-device AllToAll reshards from
head-parallel to token-parallel (core j owns tokens of batch j//2, half j%2)
for the output projection.  Host only slices/transposes weights, replicates
activations, and concatenates the 8 output shards.

Device layouts (all matmul operands bf16, fp32 PSUM accumulation):
  xT    [D, B*T]   x transposed (contraction dim on partitions)
  qT/kT [128, T]   per (local head, batch); d-order permuted so the RoPE
                   rotate-half partner sits 16 partitions away (within a
                   32-partition quadrant, reachable by DVE stream_shuffle).
                   Any consistent permutation of d leaves q.k unchanged.
  V     [T, 128]   natural d order (feeds AV matmul lhsT and out-proj order)
  S^T   [tk, tq]   scores transposed: softmax sum over the partition dim is
                   done with a ones-matmul on the PE (output is broadcast
                   across partitions for free); no max-subtraction needed
                   (logits ~ N(0,1), bounded ~ +-6).
"""

import sys

for _p in ("/opt/trn_rl_repo", "/root/.axon_site/_ro/trn_rl_repo"):
    if _p not in sys.path:
        sys.path.insert(0, _p)

import numpy as np
import ml_dtypes

BF16 = ml_dtypes.bfloat16

B = 4
D = 2048
H = 16
HD = 128
NCORES = 8
HL = 2           # heads per core
CB = D // 128    # contraction blocks
TCH = 512        # token chunk (matmul moving free dim)


def _perm128():
    """Partition order for q/k head dims: quadrant g holds dims
    [16g,16g+16) (lo) then [64+16g, 64+16g+16) (hi), so the rotate-half
    partner of partition p is p+-16 (same 32-partition quadrant)."""
    perm = np.zeros(128, np.int64)
    for p in range(128):
        g, i = divmod(p, 32)
        perm[p] = g * 16 + i if i < 16 else 64 + g * 16 + (i - 16)
    return perm


_PERM = _perm128()
_SHUF = [(i + 16) % 32 for i in range(32)]  # out[i] = in[(i+16)%32]
_SIGN = np.where(np.arange(128) % 32 < 16, -1.0, 1.0).astype(np.float32)


def build_nc(T=2048):
    import concourse.bacc as bacc
    import concourse.tile as tile
    import concourse.mybir as mybir

    f32 = mybir.dt.float32
    bf16 = mybir.dt.bfloat16
    TOK = B * T
    THALF = T // 2
    NCH = TOK // TCH          # token chunks total
    CHB = T // TCH            # token chunks per batch
    TB = T // 128             # 128-token blocks per batch
    SCALE = float(HD) ** -0.5
    Exp = mybir.ActivationFunctionType.Exp

    nc = bacc.Bacc("TRN2", target_bir_lowering=False, debug=False,
                   num_devices=NCORES)

    xT_d = nc.dram_tensor("xT", [D, TOK], bf16, kind="ExternalInput")
    wqkT_d = nc.dram_tensor("wqkT", [D, 6 * HD], bf16, kind="ExternalInput")
    woutT_d = nc.dram_tensor("woutT", [D, D], bf16, kind="ExternalInput")
    cosT_d = nc.dram_tensor("cosT", [HD, T], bf16, kind="ExternalInput")
    sinS_d = nc.dram_tensor("sinS", [HD, T], bf16, kind="ExternalInput")
    out_d = nc.dram_tensor("out", [THALF, D], f32, kind="ExternalOutput")

    xT_v = xT_d.ap().rearrange("(cb p) t -> p cb t", p=128)
    wqkT_v = wqkT_d.ap().rearrange("(cb p) f -> p cb f", p=128)
    woutT_v = woutT_d.ap().rearrange("(cb p) o -> p cb o", p=128)

    with tile.TileContext(nc) as tc:
        with (
            tc.tile_pool(name="const", bufs=1) as constp,
            tc.tile_pool(name="dram", bufs=1, space="DRAM") as dramp,
        ):
            cos_sb = constp.tile([128, T], bf16, name="cos_sb")
            sin_sb = constp.tile([128, T], bf16, name="sin_sb")
            mask_sb = constp.tile([128, 4, TCH], bf16, name="mask_sb")
            ones_sb = constp.tile([128, 128], bf16, name="ones_sb")
            nc.sync.dma_start(cos_sb[:], cosT_d[:, :])
            nc.sync.dma_start(sin_sb[:], sinS_d[:, :])
            nc.gpsimd.memset(mask_sb[:], 1.0)
            for jd in range(4):
                # keep 1.0 where  tq_rel - tk_rel - 128*jd >= 0  else 0
                nc.gpsimd.affine_select(
                    out=mask_sb[:, jd, :], in_=mask_sb[:, jd, :],
                    compare_op=mybir.AluOpType.is_ge, fill=0.0,
                    base=-128 * jd, pattern=[[1, TCH]], channel_multiplier=-1,
                )
            nc.gpsimd.memset(ones_sb[:], 1.0)

            a2a_in = dramp.tile([NCORES, HL, 128, THALF], bf16,
                                name="a2a_in")
            a2a_out = dramp.tile([NCORES, HL, 128, THALF], bf16,
                                 name="a2a_out")

            # ======== Phases 1+2 (inside qkv pool scope) ==================
            with tc.tile_pool(name="qkv", bufs=1) as qkvp:
                qT = [[qkvp.tile([128, T], bf16, name=f"qT_{hl}_{b}")
                       for b in range(B)] for hl in range(HL)]
                kT = [[qkvp.tile([128, T], bf16, name=f"kT_{hl}_{b}")
                       for b in range(B)] for hl in range(HL)]
                V = [qkvp.tile([128, TB, 2 * HD], bf16, name=f"V_{b}")
                     for b in range(B)]

                # -------- Phase 1: QKV projection + RoPE ------------------
                with (
                    tc.tile_pool(name="wqk", bufs=1) as wqkp,
                    tc.tile_pool(name="xin", bufs=2) as xp,
                    tc.tile_pool(name="ps_qk", bufs=3, space="PSUM") as psqk,
                    tc.tile_pool(name="ps_v", bufs=2, space="PSUM") as psv,
                    tc.tile_pool(name="rope", bufs=3) as ropep,
                ):
                    wqk_sb = wqkp.tile([128, CB, 6 * HD], bf16,
                                       name="wqk_sb")
                    nc.sync.dma_start(wqk_sb[:], wqkT_v)

                    for ch in range(NCH):
                        b, cc = divmod(ch, CHB)
                        t0 = cc * TCH
                        xpan = xp.tile([128, CB, TCH], bf16, tag="xpan",
                                       name=f"xpan_{ch}")
                        nc.sync.dma_start(
                            xpan[:], xT_v[:, :, ch * TCH:(ch + 1) * TCH])

                        for f in range(4):  # q_h0 q_h1 k_h0 k_h1
                            ps = psqk.tile([128, TCH], f32, tag="qk",
                                           name=f"psqk_{ch}_{f}")
                            for cb in range(CB):
                                nc.tensor.matmul(
                                    ps[:],
                                    lhsT=wqk_sb[:, cb, f * 128:(f + 1) * 128],
                                    rhs=xpan[:, cb, :],
                                    start=(cb == 0), stop=(cb == CB - 1))
                            qraw = ropep.tile([128, TCH], bf16, tag="qraw",
                                              name=f"qraw_{ch}_{f}")
                            nc.scalar.copy(qraw[:], ps[:])
                            rot = ropep.tile([128, TCH], bf16, tag="rot",
                                             name=f"rot_{ch}_{f}")
                            nc.vector.stream_shuffle(rot[:], qraw[:],
                                                     mask=_SHUF)
                            t1 = ropep.tile([128, TCH], bf16, tag="t1",
                                            name=f"t1_{ch}_{f}")
                            nc.vector.tensor_mul(
                                t1[:], qraw[:], cos_sb[:, t0:t0 + TCH])
                            nc.vector.tensor_mul(
                                rot[:], rot[:], sin_sb[:, t0:t0 + TCH])
                            dest = (qT if f < 2 else kT)[f % 2][b]
                            nc.vector.tensor_add(
                                dest[:, t0:t0 + TCH], t1[:], rot[:])

                        for tb in range(TCH // 128):  # v
                            pv = psv.tile([128, 2 * HD], f32, tag="v",
                                          name=f"psv_{ch}_{tb}")
                            for cb in range(CB):
                                nc.tensor.matmul(
                                    pv[:],
                                    lhsT=xpan[:, cb, tb * 128:(tb + 1) * 128],
                                    rhs=wqk_sb[:, cb, 4 * 128:6 * 128],
                                    start=(cb == 0), stop=(cb == CB - 1))
                            nc.scalar.copy(V[b][:, cc * 4 + tb, :], pv[:])

                # -------- Phase 2: attention ------------------------------
                with (
                    tc.tile_pool(name="ps_st", bufs=2, space="PSUM") as psst,
                    tc.tile_pool(name="ps_acc", bufs=3, space="PSUM") as psacc,
                    tc.tile_pool(name="pexp", bufs=4) as pexpp,
                    tc.tile_pool(name="onorm", bufs=4) as onp,
                ):
                    for b in range(B):
                        for hl in range(HL):
                            for tqc in range(CHB):
                                ntk = (tqc + 1) * (TCH // 128)
                                npair = ntk // 2
                                q_sl = qT[hl][b][:, tqc * TCH:(tqc + 1) * TCH]
                                av = psacc.tile([128, TCH], f32, tag="acc",
                                                name=f"av_{b}_{hl}_{tqc}")
                                ones_ps = psacc.tile(
                                    [128, TCH], f32, tag="acc",
                                    name=f"on_{b}_{hl}_{tqc}")
                                pexp_t = {}

                                def emit_pair(p, b=b, hl=hl, tqc=tqc,
                                              q_sl=q_sl, pexp_t=pexp_t):
                                    st = psst.tile(
                                        [128, 2 * TCH], f32, tag="st",
                                        name=f"st_{b}_{hl}_{tqc}_{p}")
                                    for jj in range(2):
                                        j = 2 * p + jj
                                        nc.tensor.matmul(
                                            st[:, jj * TCH:(jj + 1) * TCH],
                                            lhsT=kT[hl][b][:,
                                                           j * 128:(j + 1) * 128],
                                            rhs=q_sl,
                                            start=True, stop=True)
                                    pe = pexpp.tile(
                                        [128, 2 * TCH], bf16, tag="pexp",
                                        name=f"pe_{b}_{hl}_{tqc}_{p}")
                                    nc.scalar.activation(pe[:], st[:], Exp,
                                                         scale=SCALE)
                                    for jj in range(2):
                                        j = 2 * p + jj
                                        jd = j - (TCH // 128) * tqc
                                        if jd >= 0:
                                            sl = pe[:,
                                                    jj * TCH:(jj + 1) * TCH]
                                            nc.vector.tensor_mul(
                                                sl, sl, mask_sb[:, jd, :])
                                    pexp_t[p] = pe

                                emit_pair(0)
                                for p in range(npair):
                                    if p + 1 < npair:
                                        emit_pair(p + 1)
                                    pe = pexp_t.pop(p)
                                    for jj in range(2):
                                        j = 2 * p + jj
                                        sl = pe[:, jj * TCH:(jj + 1) * TCH]
                                        first = j == 0
                                        last = j == ntk - 1
                                        nc.tensor.matmul(
                                            ones_ps[:], lhsT=ones_sb[:],
                                            rhs=sl,
                                            start=first, stop=last,
                                            skip_group_check=True)
                                        nc.tensor.matmul(
                                            av[:],
                                            lhsT=V[b][:, j,
                                                      hl * 128:(hl + 1) * 128],
                                            rhs=sl,
                                            start=first, stop=last,
                                            skip_group_check=True)

                                recip = onp.tile([128, TCH], f32, tag="recip",
                                                 name=f"rc_{b}_{hl}_{tqc}")
                                nc.vector.reciprocal(recip[:], ones_ps[:])
                                oT = onp.tile([128, TCH], bf16, tag="oT",
                                              name=f"oT_{b}_{hl}_{tqc}")
                                nc.vector.tensor_mul(oT[:], av[:], recip[:])
                                dj = b * 2 + (tqc * TCH) // THALF
                                c0 = (tqc * TCH) % THALF
                                nc.sync.dma_start(
                                    a2a_in[dj, hl, :, c0:c0 + TCH], oT[:])

            # ======== Phase 3: AllToAll + output projection ===============
            nc.gpsimd.collective_compute(
                "AllToAll", mybir.AluOpType.bypass,
                replica_groups=[list(range(NCORES))],
                ins=[a2a_in.opt()], outs=[a2a_out.opt()],
            )
            with (
                tc.tile_pool(name="wout", bufs=2) as woutp,
                tc.tile_pool(name="attn", bufs=1) as attnp,
                tc.tile_pool(name="ps_out", bufs=3, space="PSUM") as pso,
                tc.tile_pool(name="o3", bufs=3) as o3p,
            ):
                attnall = attnp.tile([128, CB, THALF], bf16, name="attnall")
                nc.sync.dma_start(
                    attnall[:],
                    a2a_out.rearrange("i h (cb p) t -> p (i h cb) t", p=128))
                for oc in range(4):
                    half = oc // 2
                    if oc % 2 == 0:
                        wout_sb = woutp.tile([128, CB, 1024], bf16,
                                             tag="wout", name=f"wout_{half}")
                        nc.sync.dma_start(
                            wout_sb[:],
                            woutT_v[:, :, half * 1024:(half + 1) * 1024])
                    for tb in range(THALF // 128):
                        po = pso.tile([128, TCH], f32, tag="out",
                                      name=f"po_{oc}_{tb}")
                        for cb in range(CB):
                            nc.tensor.matmul(
                                po[:],
                                lhsT=attnall[:, cb, tb * 128:(tb + 1) * 128],
                                rhs=wout_sb[:, cb,
                                            (oc % 2) * TCH:(oc % 2 + 1) * TCH],
                                start=(cb == 0), stop=(cb == CB - 1))
                        ot = o3p.tile([128, TCH], f32, tag="o3",
                                      name=f"ot_{oc}_{tb}")
                        nc.scalar.copy(ot[:], po[:])
                        nc.sync.dma_start(
                            out_d[tb * 128:(tb + 1) * 128,
                                  oc * TCH:(oc + 1) * TCH],
                            ot[:])

    nc.compile()
    return nc


def prep_inputs(x, cos, sin, w_qkv, w_out, T=2048):
    """Host-side shard/layout prep. Returns in_maps for the 8 cores."""
    TOK = B * T
    xT = np.ascontiguousarray(x.reshape(TOK, D).T).astype(BF16)
    cosT = np.ascontiguousarray(cos.T[_PERM, :]).astype(BF16)
    sinS = np.ascontiguousarray(sin.T[_PERM, :] * _SIGN[:, None]).astype(BF16)
    woutT = np.ascontiguousarray(w_out.T).astype(BF16)
    in_maps = []
    for c in range(NCORES):
        rows = []
        for sec in range(2):  # q, k (perm'd)
            for hl in range(HL):
                h = 2 * c + hl
                w = w_qkv[sec * D + h * HD:sec * D + (h + 1) * HD, :]
                rows.append(w[_PERM, :])
        for hl in range(HL):  # v natural
            h = 2 * c + hl
            rows.append(w_qkv[2 * D + h * HD:2 * D + (h + 1) * HD, :])
        wqkT = np.ascontiguousarray(np.concatenate(rows, 0).T).astype(BF16)
        in_maps.append({"xT": xT, "wqkT": wqkT, "woutT": woutT,
                        "cosT": cosT, "sinS": sinS})
    return in_maps


_NC_CACHE = {}


def _get_nc(T=2048):
    if T not in _NC_CACHE:
        _NC_CACHE[T] = build_nc(T)
    return _NC_CACHE[T]


def kernel(x, cos, sin, w_qkv, w_out):
    import concourse.bass_utils as bass_utils

    T = x.shape[1]
    x = np.asarray(x, np.float32)
    cos = np.asarray(cos, np.float32)
    sin = np.asarray(sin, np.float32)
    w_qkv = np.asarray(w_qkv, np.float32)
    w_out = np.asarray(w_out, np.float32)

    nc = _get_nc(T)
    in_maps = prep_inputs(x, cos, sin, w_qkv, w_out, T)
    res = bass_utils.run_bass_kernel_spmd(nc, in_maps,
                                          core_ids=list(range(NCORES)))
    THALF = T // 2
    full = np.empty((B, T, D), np.float32)
    for j in range(NCORES):
        b, hf = divmod(j, 2)
        full[b, hf * THALF:(hf + 1) * THALF, :] = res.results[j]["out"]
    return full


# revision 9
# speedup vs baseline: 1.2005x; 1.2005x over previous
"""Causal self-attention (B=4, T=2048, D=2048, H=16, HD=128) on 8 Trainium2
NeuronCores.

Sharding: Megatron-style tensor parallel over heads for QKV projection +
attention (2 heads per core), then an on-device AllToAll reshards from
head-parallel to token-parallel (core j owns tokens of batch j//2, half j%2)
for the output projection.  Host only slices/transposes weights, replicates
activations, and concatenates the 8 output shards.

Device layouts (all matmul operands bf16, fp32 PSUM accumulation):
  xT    [D, B*T]   x transposed (contraction dim on partitions)
  qT/kT [128, T]   per (local head, batch); d-order permuted so the RoPE
                   rotate-half partner sits 16 partitions away (within a
                   32-partition quadrant, reachable by DVE stream_shuffle).
                   Any consistent permutation of d leaves q.k unchanged.
  V     [T, 128]   natural d order (feeds AV matmul lhsT and out-proj order)
  S^T   [tk, tq]   scores transposed: softmax sum over the partition dim is
                   done with a ones-matmul on the PE (output is broadcast
                   across partitions for free); no max-subtraction needed
                   (logits ~ N(0,1), bounded ~ +-6).
"""

import sys

for _p in ("/opt/trn_rl_repo", "/root/.axon_site/_ro/trn_rl_repo"):
    if _p not in sys.path:
        sys.path.insert(0, _p)

import numpy as np
import ml_dtypes

BF16 = ml_dtypes.bfloat16

B = 4
D = 2048
H = 16
HD = 128
NCORES = 8
HL = 2           # heads per core
CB = D // 128    # contraction blocks
TCH = 512        # token chunk (matmul moving free dim)


def _perm128():
    """Partition order for q/k head dims: quadrant g holds dims
    [16g,16g+16) (lo) then [64+16g, 64+16g+16) (hi), so the rotate-half
    partner of partition p is p+-16 (same 32-partition quadrant)."""
    perm = np.zeros(128, np.int64)
    for p in range(128):
        g, i = divmod(p, 32)
        perm[p] = g * 16 + i if i < 16 else 64 + g * 16 + (i - 16)
    return perm


_PERM = _perm128()
_SHUF = [(i + 16) % 32 for i in range(32)]  # out[i] = in[(i+16)%32]
_SIGN = np.where(np.arange(128) % 32 < 16, -1.0, 1.0).astype(np.float32)


def build_nc(T=2048):
    import concourse.bacc as bacc
    import concourse.tile as tile
    import concourse.mybir as mybir

    f32 = mybir.dt.float32
    bf16 = mybir.dt.bfloat16
    TOK = B * T
    THALF = T // 2
    NCH = TOK // TCH          # token chunks total
    CHB = T // TCH            # token chunks per batch
    TB = T // 128             # 128-token blocks per batch
    SCALE = float(HD) ** -0.5
    Exp = mybir.ActivationFunctionType.Exp

    assert THALF // 2 == TCH, "A2A split layout assumes T == 2048"
    nc = bacc.Bacc("TRN2", target_bir_lowering=False, debug=False,
                   num_devices=NCORES)

    xT_d = nc.dram_tensor("xT", [D, TOK], bf16, kind="ExternalInput")
    wqkT_d = nc.dram_tensor("wqkT", [D, 6 * HD], bf16, kind="ExternalInput")
    woutT_d = nc.dram_tensor("woutT", [D, D], bf16, kind="ExternalInput")
    cosT_d = nc.dram_tensor("cosT", [HD, T], bf16, kind="ExternalInput")
    sinS_d = nc.dram_tensor("sinS", [HD, T], bf16, kind="ExternalInput")
    out_d = nc.dram_tensor("out", [THALF, D], f32, kind="ExternalOutput")

    xT_v = xT_d.ap().rearrange("(cb p) t -> p cb t", p=128)
    wqkT_v = wqkT_d.ap().rearrange("(cb p) f -> p cb f", p=128)
    woutT_v = woutT_d.ap().rearrange("(cb p) o -> p cb o", p=128)

    with tile.TileContext(nc) as tc:
        with (
            tc.tile_pool(name="const", bufs=1) as constp,
            tc.tile_pool(name="dram", bufs=1, space="DRAM") as dramp,
        ):
            cos_sb = constp.tile([128, T], bf16, name="cos_sb")
            sin_sb = constp.tile([128, T], bf16, name="sin_sb")
            mask_sb = constp.tile([128, 4, TCH], bf16, name="mask_sb")
            ones_sb = constp.tile([128, 128], bf16, name="ones_sb")
            nc.sync.dma_start(cos_sb[:], cosT_d[:, :])
            nc.sync.dma_start(sin_sb[:], sinS_d[:, :])
            nc.gpsimd.memset(mask_sb[:], 1.0)
            for jd in range(4):
                # keep 1.0 where  tq_rel - tk_rel - 128*jd >= 0  else 0
                nc.gpsimd.affine_select(
                    out=mask_sb[:, jd, :], in_=mask_sb[:, jd, :],
                    compare_op=mybir.AluOpType.is_ge, fill=0.0,
                    base=-128 * jd, pattern=[[1, TCH]], channel_multiplier=-1,
                )
            nc.gpsimd.memset(ones_sb[:], 1.0)

            # Two half-sized AllToAll buffers: part 0 reshards every dest's
            # local t [0, THALF/2), part 1 the rest — so the first collective
            # overlaps the second half of attention, and the second overlaps
            # the first half of the output projection.
            TQ = THALF // 2
            a2a_in = [dramp.tile([NCORES, HL, 128, TQ], bf16,
                                 name=f"a2a_in{p}") for p in range(2)]
            a2a_out = [dramp.tile([NCORES, HL, 128, TQ], bf16,
                                  name=f"a2a_out{p}") for p in range(2)]

            # ======== Phases 1+2 (inside qkv pool scope) ==================
            with tc.tile_pool(name="qkv", bufs=1) as qkvp:
                qT = [[qkvp.tile([128, T], bf16, name=f"qT_{hl}_{b}")
                       for b in range(B)] for hl in range(HL)]
                kT = [[qkvp.tile([128, T], bf16, name=f"kT_{hl}_{b}")
                       for b in range(B)] for hl in range(HL)]
                V = [qkvp.tile([128, TB, 2 * HD], bf16, name=f"V_{b}")
                     for b in range(B)]

                # -------- Phase 1: QKV projection + RoPE ------------------
                with (
                    tc.tile_pool(name="wqk", bufs=1) as wqkp,
                    tc.tile_pool(name="xin", bufs=2) as xp,
                    tc.tile_pool(name="ps_qk", bufs=3, space="PSUM") as psqk,
                    tc.tile_pool(name="ps_v", bufs=2, space="PSUM") as psv,
                    tc.tile_pool(name="rope", bufs=3) as ropep,
                ):
                    wqk_sb = wqkp.tile([128, CB, 6 * HD], bf16,
                                       name="wqk_sb")
                    nc.sync.dma_start(wqk_sb[:, 0:CB // 2, :],
                                      wqkT_v[:, 0:CB // 2, :])
                    nc.sync.dma_start(wqk_sb[:, CB // 2:CB, :],
                                      wqkT_v[:, CB // 2:CB, :])

                    for ch in range(NCH):
                        b, cc = divmod(ch, CHB)
                        t0 = cc * TCH
                        xpan = xp.tile([128, CB, TCH], bf16, tag="xpan",
                                       name=f"xpan_{ch}")
                        for g in range(2):
                            nc.sync.dma_start(
                                xpan[:, g * CB // 2:(g + 1) * CB // 2, :],
                                xT_v[:, g * CB // 2:(g + 1) * CB // 2,
                                     ch * TCH:(ch + 1) * TCH])

                        for f in range(4):  # q_h0 q_h1 k_h0 k_h1
                            ps = psqk.tile([128, TCH], f32, tag="qk",
                                           name=f"psqk_{ch}_{f}")
                            for cb in range(CB):
                                nc.tensor.matmul(
                                    ps[:],
                                    lhsT=wqk_sb[:, cb, f * 128:(f + 1) * 128],
                                    rhs=xpan[:, cb, :],
                                    start=(cb == 0), stop=(cb == CB - 1))
                            qraw = ropep.tile([128, TCH], bf16, tag="qraw",
                                              name=f"qraw_{ch}_{f}")
                            nc.scalar.copy(qraw[:], ps[:])
                            rot = ropep.tile([128, TCH], bf16, tag="rot",
                                             name=f"rot_{ch}_{f}")
                            nc.vector.stream_shuffle(rot[:], qraw[:],
                                                     mask=_SHUF)
                            t1 = ropep.tile([128, TCH], bf16, tag="t1",
                                            name=f"t1_{ch}_{f}")
                            nc.vector.tensor_mul(
                                t1[:], qraw[:], cos_sb[:, t0:t0 + TCH])
                            nc.vector.tensor_mul(
                                rot[:], rot[:], sin_sb[:, t0:t0 + TCH])
                            dest = (qT if f < 2 else kT)[f % 2][b]
                            nc.vector.tensor_add(
                                dest[:, t0:t0 + TCH], t1[:], rot[:])

                        for tb in range(TCH // 128):  # v
                            pv = psv.tile([128, 2 * HD], f32, tag="v",
                                          name=f"psv_{ch}_{tb}")
                            for cb in range(CB):
                                nc.tensor.matmul(
                                    pv[:],
                                    lhsT=xpan[:, cb, tb * 128:(tb + 1) * 128],
                                    rhs=wqk_sb[:, cb, 4 * 128:6 * 128],
                                    start=(cb == 0), stop=(cb == CB - 1))
                            nc.scalar.copy(V[b][:, cc * 4 + tb, :], pv[:])

                # -------- Phase 2: attention ------------------------------
                with (
                    tc.tile_pool(name="ps_st", bufs=2, space="PSUM") as psst,
                    tc.tile_pool(name="ps_acc", bufs=4, space="PSUM") as psacc,
                    tc.tile_pool(name="pexp", bufs=4) as pexpp,
                    tc.tile_pool(name="onorm", bufs=4) as onp,
                ):
                    for part in range(2):
                        for tqc in range(part, CHB, 2):
                          for b in range(B):
                            for hl in range(HL):
                                ntk = (tqc + 1) * (TCH // 128)
                                npair = ntk // 2
                                q_sl = qT[hl][b][:, tqc * TCH:(tqc + 1) * TCH]
                                av = psacc.tile([128, TCH], f32, tag="acc",
                                                name=f"av_{b}_{hl}_{tqc}")
                                ones_ps = psacc.tile(
                                    [128, TCH], f32, tag="acc",
                                    name=f"on_{b}_{hl}_{tqc}")
                                pexp_t = {}

                                def emit_pair(p, b=b, hl=hl, tqc=tqc,
                                              q_sl=q_sl, pexp_t=pexp_t):
                                    st = psst.tile(
                                        [128, 2 * TCH], f32, tag="st",
                                        name=f"st_{b}_{hl}_{tqc}_{p}")
                                    for jj in range(2):
                                        j = 2 * p + jj
                                        nc.tensor.matmul(
                                            st[:, jj * TCH:(jj + 1) * TCH],
                                            lhsT=kT[hl][b][:,
                                                           j * 128:(j + 1) * 128],
                                            rhs=q_sl,
                                            start=True, stop=True)
                                    pe = pexpp.tile(
                                        [128, 2 * TCH], bf16, tag="pexp",
                                        name=f"pe_{b}_{hl}_{tqc}_{p}")
                                    nc.scalar.activation(pe[:], st[:], Exp,
                                                         scale=SCALE)
                                    for jj in range(2):
                                        j = 2 * p + jj
                                        jd = j - (TCH // 128) * tqc
                                        if jd >= 0:
                                            sl = pe[:,
                                                    jj * TCH:(jj + 1) * TCH]
                                            nc.vector.tensor_mul(
                                                sl, sl, mask_sb[:, jd, :])
                                    pexp_t[p] = pe

                                emit_pair(0)
                                for p in range(npair):
                                    if p + 1 < npair:
                                        emit_pair(p + 1)
                                    pe = pexp_t.pop(p)
                                    for jj in range(2):
                                        j = 2 * p + jj
                                        sl = pe[:, jj * TCH:(jj + 1) * TCH]
                                        first = j == 0
                                        last = j == ntk - 1
                                        nc.tensor.matmul(
                                            ones_ps[:], lhsT=ones_sb[:],
                                            rhs=sl,
                                            start=first, stop=last,
                                            skip_group_check=True)
                                        nc.tensor.matmul(
                                            av[:],
                                            lhsT=V[b][:, j,
                                                      hl * 128:(hl + 1) * 128],
                                            rhs=sl,
                                            start=first, stop=last,
                                            skip_group_check=True)

                                recip = onp.tile([128, TCH], f32, tag="recip",
                                                 name=f"rc_{b}_{hl}_{tqc}")
                                nc.vector.reciprocal(recip[:], ones_ps[:])
                                oT = onp.tile([128, TCH], bf16, tag="oT",
                                              name=f"oT_{b}_{hl}_{tqc}")
                                nc.vector.tensor_mul(oT[:], av[:], recip[:])
                                dj = b * 2 + tqc // 2
                                nc.sync.dma_start(
                                    a2a_in[part][dj, hl, :, :], oT[:])

                        # reshard this half while the other half computes
                        nc.gpsimd.collective_compute(
                            "AllToAll", mybir.AluOpType.bypass,
                            replica_groups=[list(range(NCORES))],
                            ins=[a2a_in[part].opt()],
                            outs=[a2a_out[part].opt()],
                        )

            # ======== Phase 3: output projection ==========================
            with (
                tc.tile_pool(name="wout", bufs=2) as woutp,
                tc.tile_pool(name="attn", bufs=2) as attnp,
                tc.tile_pool(name="ps_out", bufs=2, space="PSUM") as pso,
                tc.tile_pool(name="o3", bufs=3) as o3p,
            ):
                wout_sb = []
                for half in range(2):
                    w = woutp.tile([128, CB, 1024], bf16, tag="wout",
                                   name=f"wout_{half}")
                    nc.sync.dma_start(
                        w[:], woutT_v[:, :, half * 1024:(half + 1) * 1024])
                    wout_sb.append(w)
                for part in range(2):
                    attnall = attnp.tile([128, CB, TQ], bf16, tag="attnall",
                                         name=f"attnall_{part}")
                    nc.sync.dma_start(
                        attnall[:],
                        a2a_out[part].rearrange(
                            "i h (cb p) t -> p (i h cb) t", p=128))
                    for oc in range(4):
                        for tb in range(TQ // 128):
                            po = pso.tile([128, TCH], f32, tag="out",
                                          name=f"po_{part}_{oc}_{tb}")
                            for cb in range(CB):
                                nc.tensor.matmul(
                                    po[:],
                                    lhsT=attnall[:, cb,
                                                 tb * 128:(tb + 1) * 128],
                                    rhs=wout_sb[oc // 2][
                                        :, cb,
                                        (oc % 2) * TCH:(oc % 2 + 1) * TCH],
                                    start=(cb == 0), stop=(cb == CB - 1))
                            ot = o3p.tile([128, TCH], f32, tag="o3",
                                          name=f"ot_{part}_{oc}_{tb}")
                            nc.scalar.copy(ot[:], po[:])
                            nc.sync.dma_start(
                                out_d[part * TQ + tb * 128:
                                      part * TQ + (tb + 1) * 128,
                                      oc * TCH:(oc + 1) * TCH],
                                ot[:])

    nc.compile()
    return nc


def prep_inputs(x, cos, sin, w_qkv, w_out, T=2048):
    """Host-side shard/layout prep. Returns in_maps for the 8 cores."""
    TOK = B * T
    xT = np.ascontiguousarray(x.reshape(TOK, D).T).astype(BF16)
    cosT = np.ascontiguousarray(cos.T[_PERM, :]).astype(BF16)
    sinS = np.ascontiguousarray(sin.T[_PERM, :] * _SIGN[:, None]).astype(BF16)
    woutT = np.ascontiguousarray(w_out.T).astype(BF16)
    in_maps = []
    for c in range(NCORES):
        rows = []
        for sec in range(2):  # q, k (perm'd)
            for hl in range(HL):
                h = 2 * c + hl
                w = w_qkv[sec * D + h * HD:sec * D + (h + 1) * HD, :]
                rows.append(w[_PERM, :])
        for hl in range(HL):  # v natural
            h = 2 * c + hl
            rows.append(w_qkv[2 * D + h * HD:2 * D + (h + 1) * HD, :])
        wqkT = np.ascontiguousarray(np.concatenate(rows, 0).T).astype(BF16)
        in_maps.append({"xT": xT, "wqkT": wqkT, "woutT": woutT,
                        "cosT": cosT, "sinS": sinS})
    return in_maps


_NC_CACHE = {}


def _get_nc(T=2048):
    if T not in _NC_CACHE:
        _NC_CACHE[T] = build_nc(T)
    return _NC_CACHE[T]


def kernel(x, cos, sin, w_qkv, w_out):
    import concourse.bass_utils as bass_utils

    T = x.shape[1]
    x = np.asarray(x, np.float32)
    cos = np.asarray(cos, np.float32)
    sin = np.asarray(sin, np.float32)
    w_qkv = np.asarray(w_qkv, np.float32)
    w_out = np.asarray(w_out, np.float32)

    nc = _get_nc(T)
    in_maps = prep_inputs(x, cos, sin, w_qkv, w_out, T)
    res = bass_utils.run_bass_kernel_spmd(nc, in_maps,
                                          core_ids=list(range(NCORES)))
    THALF = T // 2
    full = np.empty((B, T, D), np.float32)
    for j in range(NCORES):
        b, hf = divmod(j, 2)
        full[b, hf * THALF:(hf + 1) * THALF, :] = res.results[j]["out"]
    return full


# revision 13
# speedup vs baseline: 1.2358x; 1.0294x over previous
"""Causal self-attention (B=4, T=2048, D=2048, H=16, HD=128) on 8 Trainium2
NeuronCores.

Sharding: Megatron-style tensor parallel over heads for QKV projection +
attention (2 heads per core), then an on-device AllToAll reshards from
head-parallel to token-parallel (core j owns tokens of batch j//2, half j%2)
for the output projection.  Host only slices/transposes weights, replicates
activations, and concatenates the 8 output shards.

Device layouts (all matmul operands bf16, fp32 PSUM accumulation):
  xT    [D, B*T]   x transposed (contraction dim on partitions)
  qT/kT [128, T]   per (local head, batch); d-order permuted so the RoPE
                   rotate-half partner sits 16 partitions away (within a
                   32-partition quadrant, reachable by DVE stream_shuffle).
                   Any consistent permutation of d leaves q.k unchanged.
  V     [T, 128]   natural d order (feeds AV matmul lhsT and out-proj order)
  S^T   [tk, tq]   scores transposed: softmax sum over the partition dim is
                   done with a ones-matmul on the PE (output is broadcast
                   across partitions for free); no max-subtraction needed
                   (logits ~ N(0,1), bounded ~ +-6).
"""

import sys

for _p in ("/opt/trn_rl_repo", "/root/.axon_site/_ro/trn_rl_repo"):
    if _p not in sys.path:
        sys.path.insert(0, _p)

import numpy as np
import ml_dtypes

BF16 = ml_dtypes.bfloat16

B = 4
D = 2048
H = 16
HD = 128
NCORES = 8
HL = 2           # heads per core
CB = D // 128    # contraction blocks
TCH = 512        # token chunk (matmul moving free dim)


def _perm128():
    """Partition order for q/k head dims: quadrant g holds dims
    [16g,16g+16) (lo) then [64+16g, 64+16g+16) (hi), so the rotate-half
    partner of partition p is p+-16 (same 32-partition quadrant)."""
    perm = np.zeros(128, np.int64)
    for p in range(128):
        g, i = divmod(p, 32)
        perm[p] = g * 16 + i if i < 16 else 64 + g * 16 + (i - 16)
    return perm


_PERM = _perm128()
_SHUF = [(i + 16) % 32 for i in range(32)]  # out[i] = in[(i+16)%32]
_SIGN = np.where(np.arange(128) % 32 < 16, -1.0, 1.0).astype(np.float32)


def build_nc(T=2048):
    import concourse.bacc as bacc
    import concourse.tile as tile
    import concourse.mybir as mybir

    f32 = mybir.dt.float32
    bf16 = mybir.dt.bfloat16
    TOK = B * T
    THALF = T // 2
    NCH = TOK // TCH          # token chunks total
    CHB = T // TCH            # token chunks per batch
    TB = T // 128             # 128-token blocks per batch
    SCALE = float(HD) ** -0.5
    Exp = mybir.ActivationFunctionType.Exp

    assert THALF // 2 == TCH, "A2A split layout assumes T == 2048"
    nc = bacc.Bacc("TRN2", target_bir_lowering=False, debug=False,
                   num_devices=NCORES)

    xT_d = nc.dram_tensor("xT", [D, TOK], bf16, kind="ExternalInput")
    wqkT_d = nc.dram_tensor("wqkT", [D, 6 * HD], bf16, kind="ExternalInput")
    woutT_d = nc.dram_tensor("woutT", [D, D], bf16, kind="ExternalInput")
    cosT_d = nc.dram_tensor("cosT", [HD, T], bf16, kind="ExternalInput")
    sinS_d = nc.dram_tensor("sinS", [HD, T], bf16, kind="ExternalInput")
    out_d = nc.dram_tensor("out", [THALF, D], f32, kind="ExternalOutput")

    xT_v = xT_d.ap().rearrange("(cb p) t -> p cb t", p=128)
    wqkT_v = wqkT_d.ap().rearrange("(cb p) f -> p cb f", p=128)
    woutT_v = woutT_d.ap().rearrange("(cb p) o -> p cb o", p=128)

    with tile.TileContext(nc) as tc:
        with (
            tc.tile_pool(name="const", bufs=1) as constp,
            tc.tile_pool(name="dram", bufs=1, space="DRAM") as dramp,
        ):
            cos_sb = constp.tile([128, T], bf16, name="cos_sb")
            sin_sb = constp.tile([128, T], bf16, name="sin_sb")
            mask_sb = constp.tile([128, 4, TCH], bf16, name="mask_sb")
            ones_sb = constp.tile([128, 128], bf16, name="ones_sb")
            nc.gpsimd.memset(mask_sb[:], 1.0)
            for jd in range(4):
                # keep 1.0 where  tq_rel - tk_rel - 128*jd >= 0  else 0
                nc.gpsimd.affine_select(
                    out=mask_sb[:, jd, :], in_=mask_sb[:, jd, :],
                    compare_op=mybir.AluOpType.is_ge, fill=0.0,
                    base=-128 * jd, pattern=[[1, TCH]], channel_multiplier=-1,
                )
            nc.gpsimd.memset(ones_sb[:], 1.0)

            # Two half-sized AllToAll buffers: part 0 reshards every dest's
            # local t [0, THALF/2), part 1 the rest — so the first collective
            # overlaps the second half of attention, and the second overlaps
            # the first half of the output projection.
            TQ = THALF // 2
            a2a_in = [dramp.tile([NCORES, HL, 128, TQ], bf16,
                                 name=f"a2a_in{p}") for p in range(2)]
            a2a_out = [dramp.tile([NCORES, HL, 128, TQ], bf16,
                                  name=f"a2a_out{p}") for p in range(2)]

            # ======== Phases 1+2 (inside qkv pool scope) ==================
            with tc.tile_pool(name="qkv", bufs=1) as qkvp:
                qT = [[qkvp.tile([128, T], bf16, name=f"qT_{hl}_{b}")
                       for b in range(B)] for hl in range(HL)]
                kT = [[qkvp.tile([128, T], bf16, name=f"kT_{hl}_{b}")
                       for b in range(B)] for hl in range(HL)]
                V = [qkvp.tile([128, TB, 2 * HD], bf16, name=f"V_{b}")
                     for b in range(B)]

                # -------- Phase 1: QKV projection + RoPE ------------------
                with (
                    tc.tile_pool(name="wqk", bufs=1) as wqkp,
                    tc.tile_pool(name="xin", bufs=2) as xp,
                    tc.tile_pool(name="ps_qk", bufs=3, space="PSUM") as psqk,
                    tc.tile_pool(name="ps_v", bufs=2, space="PSUM") as psv,
                    tc.tile_pool(name="rope", bufs=3) as ropep,
                ):
                    wqk_sb = wqkp.tile([128, CB, 6 * HD], bf16,
                                       name="wqk_sb")
                    nc.sync.dma_start(wqk_sb[:, 0:CB // 2, :],
                                      wqkT_v[:, 0:CB // 2, :])
                    nc.sync.dma_start(wqk_sb[:, CB // 2:CB, :],
                                      wqkT_v[:, CB // 2:CB, :])

                    for ch in range(NCH):
                        b, cc = divmod(ch, CHB)
                        t0 = cc * TCH
                        xpan = xp.tile([128, CB, TCH], bf16, tag="xpan",
                                       name=f"xpan_{ch}")
                        for g in range(2):
                            nc.sync.dma_start(
                                xpan[:, g * CB // 2:(g + 1) * CB // 2, :],
                                xT_v[:, g * CB // 2:(g + 1) * CB // 2,
                                     ch * TCH:(ch + 1) * TCH])
                        if ch == 0:
                            # behind the critical first weight/x loads
                            nc.sync.dma_start(cos_sb[:], cosT_d[:, :])
                            nc.sync.dma_start(sin_sb[:], sinS_d[:, :])

                        for f in range(4):  # q_h0 q_h1 k_h0 k_h1
                            ps = psqk.tile([128, TCH], f32, tag="qk",
                                           name=f"psqk_{ch}_{f}")
                            for cb in range(CB):
                                nc.tensor.matmul(
                                    ps[:],
                                    lhsT=wqk_sb[:, cb, f * 128:(f + 1) * 128],
                                    rhs=xpan[:, cb, :],
                                    start=(cb == 0), stop=(cb == CB - 1))
                            qraw = ropep.tile([128, TCH], bf16, tag="qraw",
                                              name=f"qraw_{ch}_{f}")
                            nc.scalar.copy(qraw[:], ps[:])
                            rot = ropep.tile([128, TCH], bf16, tag="rot",
                                             name=f"rot_{ch}_{f}")
                            nc.vector.stream_shuffle(rot[:], qraw[:],
                                                     mask=_SHUF)
                            t1 = ropep.tile([128, TCH], bf16, tag="t1",
                                            name=f"t1_{ch}_{f}")
                            nc.vector.tensor_mul(
                                t1[:], qraw[:], cos_sb[:, t0:t0 + TCH])
                            nc.vector.tensor_mul(
                                rot[:], rot[:], sin_sb[:, t0:t0 + TCH])
                            dest = (qT if f < 2 else kT)[f % 2][b]
                            nc.vector.tensor_add(
                                dest[:, t0:t0 + TCH], t1[:], rot[:])

                        for tb in range(TCH // 128):  # v
                            pv = psv.tile([128, 2 * HD], f32, tag="v",
                                          name=f"psv_{ch}_{tb}")
                            for cb in range(CB):
                                nc.tensor.matmul(
                                    pv[:],
                                    lhsT=xpan[:, cb, tb * 128:(tb + 1) * 128],
                                    rhs=wqk_sb[:, cb, 4 * 128:6 * 128],
                                    start=(cb == 0), stop=(cb == CB - 1))
                            nc.scalar.copy(V[b][:, cc * 4 + tb, :], pv[:])

                # -------- Phase 2: attention ------------------------------
                with (
                    tc.tile_pool(name="ps_st", bufs=2, space="PSUM") as psst,
                    tc.tile_pool(name="ps_acc", bufs=4, space="PSUM") as psacc,
                    tc.tile_pool(name="pexp", bufs=4) as pexpp,
                    tc.tile_pool(name="onorm", bufs=4) as onp,
                ):
                    for part in range(2):
                        for tqc in range(part, CHB, 2):
                          for b in range(B):
                            for hl in range(HL):
                                ntk = (tqc + 1) * (TCH // 128)
                                npair = ntk // 2
                                q_sl = qT[hl][b][:, tqc * TCH:(tqc + 1) * TCH]
                                av = psacc.tile([128, TCH], f32, tag="acc",
                                                name=f"av_{b}_{hl}_{tqc}")
                                ones_ps = psacc.tile(
                                    [128, TCH], f32, tag="acc",
                                    name=f"on_{b}_{hl}_{tqc}")
                                pexp_t = {}

                                def emit_pair(p, b=b, hl=hl, tqc=tqc,
                                              q_sl=q_sl, pexp_t=pexp_t):
                                    st = psst.tile(
                                        [128, 2 * TCH], f32, tag="st",
                                        name=f"st_{b}_{hl}_{tqc}_{p}")
                                    for jj in range(2):
                                        j = 2 * p + jj
                                        nc.tensor.matmul(
                                            st[:, jj * TCH:(jj + 1) * TCH],
                                            lhsT=kT[hl][b][:,
                                                           j * 128:(j + 1) * 128],
                                            rhs=q_sl,
                                            start=True, stop=True)
                                    pe = pexpp.tile(
                                        [128, 2 * TCH], bf16, tag="pexp",
                                        name=f"pe_{b}_{hl}_{tqc}_{p}")
                                    nc.scalar.activation(pe[:], st[:], Exp,
                                                         scale=SCALE)
                                    for jj in range(2):
                                        j = 2 * p + jj
                                        jd = j - (TCH // 128) * tqc
                                        if jd >= 0:
                                            sl = pe[:,
                                                    jj * TCH:(jj + 1) * TCH]
                                            nc.vector.tensor_mul(
                                                sl, sl, mask_sb[:, jd, :])
                                    pexp_t[p] = pe

                                emit_pair(0)
                                for p in range(npair):
                                    if p + 1 < npair:
                                        emit_pair(p + 1)
                                    pe = pexp_t.pop(p)
                                    for jj in range(2):
                                        j = 2 * p + jj
                                        sl = pe[:, jj * TCH:(jj + 1) * TCH]
                                        first = j == 0
                                        last = j == ntk - 1
                                        nc.tensor.matmul(
                                            ones_ps[:], lhsT=ones_sb[:],
                                            rhs=sl,
                                            start=first, stop=last,
                                            skip_group_check=True)
                                        nc.tensor.matmul(
                                            av[:],
                                            lhsT=V[b][:, j,
                                                      hl * 128:(hl + 1) * 128],
                                            rhs=sl,
                                            start=first, stop=last,
                                            skip_group_check=True)

                                recip = onp.tile([128, TCH], f32, tag="recip",
                                                 name=f"rc_{b}_{hl}_{tqc}")
                                nc.vector.reciprocal_approx_fast(
                                    recip[:], ones_ps[:])
                                oT = onp.tile([128, TCH], bf16, tag="oT",
                                              name=f"oT_{b}_{hl}_{tqc}")
                                nc.vector.tensor_mul(oT[:], av[:], recip[:])
                                dj = b * 2 + tqc // 2
                                nc.sync.dma_start(
                                    a2a_in[part][dj, hl, :, :], oT[:])

                        # reshard this half while the other half computes
                        nc.gpsimd.collective_compute(
                            "AllToAll", mybir.AluOpType.bypass,
                            replica_groups=[list(range(NCORES))],
                            ins=[a2a_in[part].opt()],
                            outs=[a2a_out[part].opt()],
                        )

            # ======== Phase 3: output projection ==========================
            with (
                tc.tile_pool(name="wout", bufs=4) as woutp,
                tc.tile_pool(name="attn", bufs=2) as attnp,
                tc.tile_pool(name="ps_out", bufs=2, space="PSUM") as pso,
                tc.tile_pool(name="o3", bufs=3) as o3p,
            ):
                # attnall part 0 first: it is the critical load after the
                # first collective; wout chunks stream in behind it.
                attnall_t = []
                attnall = attnp.tile([128, CB, TQ], bf16, tag="attnall",
                                     name="attnall_0")
                nc.sync.dma_start(
                    attnall[:],
                    a2a_out[0].rearrange("i h (cb p) t -> p (i h cb) t",
                                         p=128))
                attnall_t.append(attnall)
                wout_sb = []
                for oc in range(4):
                    w = woutp.tile([128, CB, TCH], bf16, tag="wout",
                                   name=f"wout_{oc}")
                    nc.sync.dma_start(
                        w[:], woutT_v[:, :, oc * TCH:(oc + 1) * TCH])
                    wout_sb.append(w)
                attnall = attnp.tile([128, CB, TQ], bf16, tag="attnall",
                                     name="attnall_1")
                nc.sync.dma_start(
                    attnall[:],
                    a2a_out[1].rearrange("i h (cb p) t -> p (i h cb) t",
                                         p=128))
                attnall_t.append(attnall)

                for part in range(2):
                    attnall = attnall_t[part]
                    for oc in range(4):
                        for tb in range(TQ // 128):
                            po = pso.tile([128, TCH], f32, tag="out",
                                          name=f"po_{part}_{oc}_{tb}")
                            for cb in range(CB):
                                nc.tensor.matmul(
                                    po[:],
                                    lhsT=attnall[:, cb,
                                                 tb * 128:(tb + 1) * 128],
                                    rhs=wout_sb[oc][:, cb, :],
                                    start=(cb == 0), stop=(cb == CB - 1))
                            ot = o3p.tile([128, TCH], f32, tag="o3",
                                          name=f"ot_{part}_{oc}_{tb}")
                            nc.scalar.copy(ot[:], po[:])
                            nc.sync.dma_start(
                                out_d[part * TQ + tb * 128:
                                      part * TQ + (tb + 1) * 128,
                                      oc * TCH:(oc + 1) * TCH],
                                ot[:])

    nc.compile()
    return nc


def prep_inputs(x, cos, sin, w_qkv, w_out, T=2048):
    """Host-side shard/layout prep. Returns in_maps for the 8 cores."""
    TOK = B * T
    xT = np.ascontiguousarray(x.reshape(TOK, D).T).astype(BF16)
    cosT = np.ascontiguousarray(cos.T[_PERM, :]).astype(BF16)
    sinS = np.ascontiguousarray(sin.T[_PERM, :] * _SIGN[:, None]).astype(BF16)
    woutT = np.ascontiguousarray(w_out.T).astype(BF16)
    in_maps = []
    for c in range(NCORES):
        rows = []
        for sec in range(2):  # q, k (perm'd)
            for hl in range(HL):
                h = 2 * c + hl
                w = w_qkv[sec * D + h * HD:sec * D + (h + 1) * HD, :]
                rows.append(w[_PERM, :])
        for hl in range(HL):  # v natural
            h = 2 * c + hl
            rows.append(w_qkv[2 * D + h * HD:2 * D + (h + 1) * HD, :])
        wqkT = np.ascontiguousarray(np.concatenate(rows, 0).T).astype(BF16)
        in_maps.append({"xT": xT, "wqkT": wqkT, "woutT": woutT,
                        "cosT": cosT, "sinS": sinS})
    return in_maps


_NC_CACHE = {}


def _get_nc(T=2048):
    if T not in _NC_CACHE:
        _NC_CACHE[T] = build_nc(T)
    return _NC_CACHE[T]


def kernel(x, cos, sin, w_qkv, w_out):
    import concourse.bass_utils as bass_utils

    T = x.shape[1]
    x = np.asarray(x, np.float32)
    cos = np.asarray(cos, np.float32)
    sin = np.asarray(sin, np.float32)
    w_qkv = np.asarray(w_qkv, np.float32)
    w_out = np.asarray(w_out, np.float32)

    nc = _get_nc(T)
    in_maps = prep_inputs(x, cos, sin, w_qkv, w_out, T)
    res = bass_utils.run_bass_kernel_spmd(nc, in_maps,
                                          core_ids=list(range(NCORES)))
    THALF = T // 2
    full = np.empty((B, T, D), np.float32)
    for j in range(NCORES):
        b, hf = divmod(j, 2)
        full[b, hf * THALF:(hf + 1) * THALF, :] = res.results[j]["out"]
    return full


# revision 15
# speedup vs baseline: 1.2840x; 1.0390x over previous
"""Causal self-attention (B=4, T=2048, D=2048, H=16, HD=128) on 8 Trainium2
NeuronCores.

Sharding: Megatron-style tensor parallel over heads for QKV projection +
attention (2 heads per core), then an on-device AllToAll reshards from
head-parallel to token-parallel (core j owns tokens of batch j//2, half j%2)
for the output projection.  Host only slices/transposes weights, replicates
activations, and concatenates the 8 output shards.

Device layouts (all matmul operands bf16, fp32 PSUM accumulation):
  xT    [D, B*T]   x transposed (contraction dim on partitions)
  qT/kT [128, T]   per (local head, batch); d-order permuted so the RoPE
                   rotate-half partner sits 16 partitions away (within a
                   32-partition quadrant, reachable by DVE stream_shuffle).
                   Any consistent permutation of d leaves q.k unchanged.
  V     [T, 128]   natural d order (feeds AV matmul lhsT and out-proj order)
  S^T   [tk, tq]   scores transposed: softmax sum over the partition dim is
                   done with a ones-matmul on the PE (output is broadcast
                   across partitions for free); no max-subtraction needed
                   (logits ~ N(0,1), bounded ~ +-6).
"""

import sys

for _p in ("/opt/trn_rl_repo", "/root/.axon_site/_ro/trn_rl_repo"):
    if _p not in sys.path:
        sys.path.insert(0, _p)

import numpy as np
import ml_dtypes

BF16 = ml_dtypes.bfloat16

B = 4
D = 2048
H = 16
HD = 128
NCORES = 8
HL = 2           # heads per core
CB = D // 128    # contraction blocks
TCH = 512        # token chunk (matmul moving free dim)


def _perm128():
    """Partition order for q/k head dims: quadrant g holds dims
    [16g,16g+16) (lo) then [64+16g, 64+16g+16) (hi), so the rotate-half
    partner of partition p is p+-16 (same 32-partition quadrant)."""
    perm = np.zeros(128, np.int64)
    for p in range(128):
        g, i = divmod(p, 32)
        perm[p] = g * 16 + i if i < 16 else 64 + g * 16 + (i - 16)
    return perm


_PERM = _perm128()
_SHUF = [(i + 16) % 32 for i in range(32)]  # out[i] = in[(i+16)%32]
_SIGN = np.where(np.arange(128) % 32 < 16, -1.0, 1.0).astype(np.float32)


def build_nc(T=2048):
    import concourse.bacc as bacc
    import concourse.tile as tile
    import concourse.mybir as mybir

    f32 = mybir.dt.float32
    bf16 = mybir.dt.bfloat16
    TOK = B * T
    THALF = T // 2
    NCH = TOK // TCH          # token chunks total
    CHB = T // TCH            # token chunks per batch
    TB = T // 128             # 128-token blocks per batch
    SCALE = float(HD) ** -0.5
    Exp = mybir.ActivationFunctionType.Exp

    assert THALF // 2 == TCH, "A2A split layout assumes T == 2048"
    nc = bacc.Bacc("TRN2", target_bir_lowering=False, debug=False,
                   num_devices=NCORES)

    xT_d = nc.dram_tensor("xT", [D, TOK], bf16, kind="ExternalInput")
    wqkT_d = nc.dram_tensor("wqkT", [D, 6 * HD], bf16, kind="ExternalInput")
    woutT_d = nc.dram_tensor("woutT", [D, D], bf16, kind="ExternalInput")
    cosT_d = nc.dram_tensor("cosT", [HD, T], bf16, kind="ExternalInput")
    sinS_d = nc.dram_tensor("sinS", [HD, T], bf16, kind="ExternalInput")
    out_d = nc.dram_tensor("out", [THALF, D], f32, kind="ExternalOutput")

    xT_v = xT_d.ap().rearrange("(cb p) t -> p cb t", p=128)
    wqkT_v = wqkT_d.ap().rearrange("(cb p) f -> p cb f", p=128)
    woutT_v = woutT_d.ap().rearrange("(cb p) o -> p cb o", p=128)

    with tile.TileContext(nc) as tc:
        with (
            tc.tile_pool(name="const", bufs=1) as constp,
            tc.tile_pool(name="dram", bufs=1, space="DRAM") as dramp,
        ):
            cos_sb = constp.tile([128, T], bf16, name="cos_sb")
            sin_sb = constp.tile([128, T], bf16, name="sin_sb")
            mask_sb = constp.tile([128, 4, TCH], bf16, name="mask_sb")
            ones_sb = constp.tile([128, 128], bf16, name="ones_sb")
            nc.gpsimd.memset(mask_sb[:], 1.0)
            for jd in range(4):
                # keep 1.0 where  tq_rel - tk_rel - 128*jd >= 0  else 0
                nc.gpsimd.affine_select(
                    out=mask_sb[:, jd, :], in_=mask_sb[:, jd, :],
                    compare_op=mybir.AluOpType.is_ge, fill=0.0,
                    base=-128 * jd, pattern=[[1, TCH]], channel_multiplier=-1,
                )
            nc.gpsimd.memset(ones_sb[:], 1.0)

            # Two half-sized AllToAll buffers: part 0 reshards every dest's
            # local t [0, THALF/2), part 1 the rest — so the first collective
            # overlaps the second half of attention, and the second overlaps
            # the first half of the output projection.
            TQ = THALF // 2
            a2a_in = [dramp.tile([NCORES, HL, 128, TQ], bf16,
                                 name=f"a2a_in{p}") for p in range(2)]
            a2a_out = [dramp.tile([NCORES, HL, 128, TQ], bf16,
                                  name=f"a2a_out{p}") for p in range(2)]

            # ======== Phases 1+2 (inside qkv pool scope) ==================
            with tc.tile_pool(name="qkv", bufs=1) as qkvp:
                qT = [[qkvp.tile([128, T], bf16, name=f"qT_{hl}_{b}")
                       for b in range(B)] for hl in range(HL)]
                kT = [[qkvp.tile([128, T], bf16, name=f"kT_{hl}_{b}")
                       for b in range(B)] for hl in range(HL)]
                V = [qkvp.tile([128, TB, 2 * HD], bf16, name=f"V_{b}")
                     for b in range(B)]

                # -------- Phase 1: QKV projection + RoPE ------------------
                with (
                    tc.tile_pool(name="wqk", bufs=1) as wqkp,
                    tc.tile_pool(name="xin", bufs=2) as xp,
                    tc.tile_pool(name="ps_qk", bufs=3, space="PSUM") as psqk,
                    tc.tile_pool(name="ps_v", bufs=2, space="PSUM") as psv,
                    tc.tile_pool(name="rope", bufs=3) as ropep,
                ):
                    wqk_sb = wqkp.tile([128, CB, 6 * HD], bf16,
                                       name="wqk_sb")
                    nc.sync.dma_start(wqk_sb[:, 0:CB // 2, :],
                                      wqkT_v[:, 0:CB // 2, :])
                    nc.sync.dma_start(wqk_sb[:, CB // 2:CB, :],
                                      wqkT_v[:, CB // 2:CB, :])

                    for ch in range(NCH):
                        b, cc = divmod(ch, CHB)
                        t0 = cc * TCH
                        xpan = xp.tile([128, CB, TCH], bf16, tag="xpan",
                                       name=f"xpan_{ch}")
                        for g in range(2):
                            nc.sync.dma_start(
                                xpan[:, g * CB // 2:(g + 1) * CB // 2, :],
                                xT_v[:, g * CB // 2:(g + 1) * CB // 2,
                                     ch * TCH:(ch + 1) * TCH])
                        if ch == 0:
                            # behind the critical first weight/x loads
                            nc.sync.dma_start(cos_sb[:], cosT_d[:, :])
                            nc.sync.dma_start(sin_sb[:], sinS_d[:, :])

                        for f in range(4):  # q_h0 q_h1 k_h0 k_h1
                            ps = psqk.tile([128, TCH], f32, tag="qk",
                                           name=f"psqk_{ch}_{f}")
                            for cb in range(CB):
                                nc.tensor.matmul(
                                    ps[:],
                                    lhsT=wqk_sb[:, cb, f * 128:(f + 1) * 128],
                                    rhs=xpan[:, cb, :],
                                    start=(cb == 0), stop=(cb == CB - 1))
                            qraw = ropep.tile([128, TCH], bf16, tag="qraw",
                                              name=f"qraw_{ch}_{f}")
                            nc.scalar.copy(qraw[:], ps[:])
                            rot = ropep.tile([128, TCH], bf16, tag="rot",
                                             name=f"rot_{ch}_{f}")
                            nc.vector.stream_shuffle(rot[:], qraw[:],
                                                     mask=_SHUF)
                            t1 = ropep.tile([128, TCH], bf16, tag="t1",
                                            name=f"t1_{ch}_{f}")
                            nc.vector.tensor_mul(
                                t1[:], qraw[:], cos_sb[:, t0:t0 + TCH])
                            nc.vector.tensor_mul(
                                rot[:], rot[:], sin_sb[:, t0:t0 + TCH])
                            dest = (qT if f < 2 else kT)[f % 2][b]
                            nc.vector.tensor_add(
                                dest[:, t0:t0 + TCH], t1[:], rot[:])

                        for tb in range(TCH // 128):  # v
                            pv = psv.tile([128, 2 * HD], f32, tag="v",
                                          name=f"psv_{ch}_{tb}")
                            for cb in range(CB):
                                nc.tensor.matmul(
                                    pv[:],
                                    lhsT=xpan[:, cb, tb * 128:(tb + 1) * 128],
                                    rhs=wqk_sb[:, cb, 4 * 128:6 * 128],
                                    start=(cb == 0), stop=(cb == CB - 1))
                            nc.scalar.copy(V[b][:, cc * 4 + tb, :], pv[:])

                # -------- Phase 2: attention ------------------------------
                attnall_t = []
                wout_pre = {}
                with (
                    tc.tile_pool(name="attn", bufs=2) as attnp,
                    tc.tile_pool(name="wout", bufs=2) as woutp,
                ):
                 with (
                    tc.tile_pool(name="ps_st", bufs=2, space="PSUM") as psst,
                    tc.tile_pool(name="ps_acc", bufs=4, space="PSUM") as psacc,
                    tc.tile_pool(name="pexp", bufs=3) as pexpp,
                    tc.tile_pool(name="onorm", bufs=3) as onp,
                 ):
                    for part in range(2):
                        for tqc in range(part, CHB, 2):
                          for b in range(B):
                            for hl in range(HL):
                                ntk = (tqc + 1) * (TCH // 128)
                                npair = ntk // 2
                                q_sl = qT[hl][b][:, tqc * TCH:(tqc + 1) * TCH]
                                av = psacc.tile([128, TCH], f32, tag="acc",
                                                name=f"av_{b}_{hl}_{tqc}")
                                ones_ps = psacc.tile(
                                    [128, TCH], f32, tag="acc",
                                    name=f"on_{b}_{hl}_{tqc}")
                                pexp_t = {}

                                def emit_pair(p, b=b, hl=hl, tqc=tqc,
                                              q_sl=q_sl, pexp_t=pexp_t):
                                    st = psst.tile(
                                        [128, 2 * TCH], f32, tag="st",
                                        name=f"st_{b}_{hl}_{tqc}_{p}")
                                    for jj in range(2):
                                        j = 2 * p + jj
                                        nc.tensor.matmul(
                                            st[:, jj * TCH:(jj + 1) * TCH],
                                            lhsT=kT[hl][b][:,
                                                           j * 128:(j + 1) * 128],
                                            rhs=q_sl,
                                            start=True, stop=True)
                                    pe = pexpp.tile(
                                        [128, 2 * TCH], bf16, tag="pexp",
                                        name=f"pe_{b}_{hl}_{tqc}_{p}")
                                    nc.scalar.activation(pe[:], st[:], Exp,
                                                         scale=SCALE)
                                    for jj in range(2):
                                        j = 2 * p + jj
                                        jd = j - (TCH // 128) * tqc
                                        if jd >= 0:
                                            sl = pe[:,
                                                    jj * TCH:(jj + 1) * TCH]
                                            nc.vector.tensor_mul(
                                                sl, sl, mask_sb[:, jd, :])
                                    pexp_t[p] = pe

                                emit_pair(0)
                                for p in range(npair):
                                    if p + 1 < npair:
                                        emit_pair(p + 1)
                                    pe = pexp_t.pop(p)
                                    for jj in range(2):
                                        j = 2 * p + jj
                                        sl = pe[:, jj * TCH:(jj + 1) * TCH]
                                        first = j == 0
                                        last = j == ntk - 1
                                        nc.tensor.matmul(
                                            ones_ps[:], lhsT=ones_sb[:],
                                            rhs=sl,
                                            start=first, stop=last,
                                            skip_group_check=True)
                                        nc.tensor.matmul(
                                            av[:],
                                            lhsT=V[b][:, j,
                                                      hl * 128:(hl + 1) * 128],
                                            rhs=sl,
                                            start=first, stop=last,
                                            skip_group_check=True)

                                recip = onp.tile([128, TCH], f32, tag="recip",
                                                 name=f"rc_{b}_{hl}_{tqc}")
                                nc.vector.reciprocal_approx_fast(
                                    recip[:], ones_ps[:])
                                oT = onp.tile([128, TCH], bf16, tag="oT",
                                              name=f"oT_{b}_{hl}_{tqc}")
                                nc.vector.tensor_mul(oT[:], av[:], recip[:])
                                dj = b * 2 + tqc // 2
                                nc.sync.dma_start(
                                    a2a_in[part][dj, hl, :, :], oT[:])

                        # reshard this half while the other half computes /
                        # the output projection runs
                        nc.gpsimd.collective_compute(
                            "AllToAll", mybir.AluOpType.bypass,
                            replica_groups=[list(range(NCORES))],
                            ins=[a2a_in[part].opt()],
                            outs=[a2a_out[part].opt()],
                        )
                        # critical post-collective load on the (idle) gpsimd
                        # queue so it is not stuck behind Sync-queue DMAs
                        attnall = attnp.tile([128, CB, TQ], bf16,
                                             tag="attnall",
                                             name=f"attnall_{part}")
                        nc.gpsimd.dma_start(
                            attnall[:],
                            a2a_out[part].rearrange(
                                "i h (cb p) t -> p (i h cb) t", p=128))
                        attnall_t.append(attnall)
                        if part == 0:
                            for oc in range(2):
                                w = woutp.tile([128, CB, TCH], bf16,
                                               tag="wout",
                                               name=f"wout_0_{oc}")
                                nc.gpsimd.dma_start(
                                    w[:],
                                    woutT_v[:, :, oc * TCH:(oc + 1) * TCH])
                                wout_pre[(0, oc)] = w

                 # ======== Phase 3: output projection ======================
                 with (
                    tc.tile_pool(name="ps_out", bufs=2, space="PSUM") as pso,
                    tc.tile_pool(name="o3", bufs=3) as o3p,
                 ):
                    last_mm = None
                    first_mm_p1 = None
                    for part in range(2):
                        attnall = attnall_t[part]
                        for oc in range(4):
                            if (part, oc) in wout_pre:
                                w = wout_pre[(part, oc)]
                            else:
                                w = woutp.tile([128, CB, TCH], bf16,
                                               tag="wout",
                                               name=f"wout_{part}_{oc}")
                                nc.sync.dma_start(
                                    w[:],
                                    woutT_v[:, :, oc * TCH:(oc + 1) * TCH])
                            for tb in range(TQ // 128):
                                po = pso.tile([128, TCH], f32, tag="out",
                                              name=f"po_{part}_{oc}_{tb}")
                                for cb in range(CB):
                                    mm = nc.tensor.matmul(
                                        po[:],
                                        lhsT=attnall[:, cb,
                                                     tb * 128:(tb + 1) * 128],
                                        rhs=w[:, cb, :],
                                        start=(cb == 0),
                                        stop=(cb == CB - 1))
                                    if part == 1 and first_mm_p1 is None:
                                        first_mm_p1 = mm
                                    if part == 0:
                                        last_mm = mm
                                ot = o3p.tile([128, TCH], f32, tag="o3",
                                              name=f"ot_{part}_{oc}_{tb}")
                                nc.scalar.copy(ot[:], po[:])
                                nc.sync.dma_start(
                                    out_d[part * TQ + tb * 128:
                                          part * TQ + (tb + 1) * 128,
                                          oc * TCH:(oc + 1) * TCH],
                                    ot[:])
                    # keep the two out-proj halves in emission order on the
                    # PE so part 1 (gated on the second collective) cannot
                    # starve part 0's remaining matmuls
                    tile.add_dep_helper(
                        first_mm_p1.ins, last_mm.ins, sync=False,
                        reason="outproj part order")

    nc.compile()
    return nc


def prep_inputs(x, cos, sin, w_qkv, w_out, T=2048):
    """Host-side shard/layout prep. Returns in_maps for the 8 cores."""
    TOK = B * T
    xT = np.ascontiguousarray(x.reshape(TOK, D).T).astype(BF16)
    cosT = np.ascontiguousarray(cos.T[_PERM, :]).astype(BF16)
    sinS = np.ascontiguousarray(sin.T[_PERM, :] * _SIGN[:, None]).astype(BF16)
    woutT = np.ascontiguousarray(w_out.T).astype(BF16)
    in_maps = []
    for c in range(NCORES):
        rows = []
        for sec in range(2):  # q, k (perm'd)
            for hl in range(HL):
                h = 2 * c + hl
                w = w_qkv[sec * D + h * HD:sec * D + (h + 1) * HD, :]
                rows.append(w[_PERM, :])
        for hl in range(HL):  # v natural
            h = 2 * c + hl
            rows.append(w_qkv[2 * D + h * HD:2 * D + (h + 1) * HD, :])
        wqkT = np.ascontiguousarray(np.concatenate(rows, 0).T).astype(BF16)
        in_maps.append({"xT": xT, "wqkT": wqkT, "woutT": woutT,
                        "cosT": cosT, "sinS": sinS})
    return in_maps


_NC_CACHE = {}


def _get_nc(T=2048):
    if T not in _NC_CACHE:
        _NC_CACHE[T] = build_nc(T)
    return _NC_CACHE[T]


def kernel(x, cos, sin, w_qkv, w_out):
    import concourse.bass_utils as bass_utils

    T = x.shape[1]
    x = np.asarray(x, np.float32)
    cos = np.asarray(cos, np.float32)
    sin = np.asarray(sin, np.float32)
    w_qkv = np.asarray(w_qkv, np.float32)
    w_out = np.asarray(w_out, np.float32)

    nc = _get_nc(T)
    in_maps = prep_inputs(x, cos, sin, w_qkv, w_out, T)
    res = bass_utils.run_bass_kernel_spmd(nc, in_maps,
                                          core_ids=list(range(NCORES)))
    THALF = T // 2
    full = np.empty((B, T, D), np.float32)
    for j in range(NCORES):
        b, hf = divmod(j, 2)
        full[b, hf * THALF:(hf + 1) * THALF, :] = res.results[j]["out"]
    return full


# revision 17
# speedup vs baseline: 1.3090x; 1.0194x over previous
"""Causal self-attention (B=4, T=2048, D=2048, H=16, HD=128) on 8 Trainium2
NeuronCores.

Sharding: Megatron-style tensor parallel over heads for QKV projection +
attention (2 heads per core), then on-device AllToAlls reshard from
head-parallel to token-parallel (core j owns tokens of batch j//2, half j%2)
for the output projection.  Host only slices/transposes weights, replicates
activations, and concatenates the 8 output shards.

Device layouts (all matmul operands bf16, fp32 PSUM accumulation):
  xT    [D, B*T]   x transposed (contraction dim on partitions)
  qT/kT [128, T]   per (local head, batch); d-order permuted so the RoPE
                   rotate-half partner sits 16 partitions away (within a
                   32-partition quadrant, reachable by DVE stream_shuffle).
                   Any consistent permutation of d leaves q.k unchanged.
  V     [T, 128]   natural d order (feeds AV matmul lhsT and out-proj order)
  S^T   [tk, tq]   scores transposed: the softmax sum over the partition dim
                   is a ones-matmul on the PE (output rows are the broadcast
                   sums for free); no max-subtraction needed (logits ~
                   N(0,1), bounded ~ +-6, exp can't overflow).

The attention loop runs tq-half 0 (even 512-token chunks) then half 1, with
one AllToAll per (half, head) issued as soon as that head's chunks finish —
all four collectives overlap the remaining attention / output projection.
"""

import sys

for _p in ("/opt/trn_rl_repo", "/root/.axon_site/_ro/trn_rl_repo"):
    if _p not in sys.path:
        sys.path.insert(0, _p)

import numpy as np
import ml_dtypes

BF16 = ml_dtypes.bfloat16

B = 4
D = 2048
H = 16
HD = 128
NCORES = 8
HL = 2           # heads per core
CB = D // 128    # contraction blocks
TCH = 512        # token chunk (matmul moving free dim)


def _perm128():
    """Partition order for q/k head dims: quadrant g holds dims
    [16g,16g+16) (lo) then [64+16g, 64+16g+16) (hi), so the rotate-half
    partner of partition p is p+-16 (same 32-partition quadrant)."""
    perm = np.zeros(128, np.int64)
    for p in range(128):
        g, i = divmod(p, 32)
        perm[p] = g * 16 + i if i < 16 else 64 + g * 16 + (i - 16)
    return perm


_PERM = _perm128()
_SHUF = [(i + 16) % 32 for i in range(32)]  # out[i] = in[(i+16)%32]
_SIGN = np.where(np.arange(128) % 32 < 16, -1.0, 1.0).astype(np.float32)


def build_nc(T=2048):
    import concourse.bacc as bacc
    import concourse.tile as tile
    import concourse.mybir as mybir

    f32 = mybir.dt.float32
    bf16 = mybir.dt.bfloat16
    TOK = B * T
    THALF = T // 2
    TQ = THALF // 2           # tokens per (core, a2a part)
    NCH = TOK // TCH          # token chunks total
    CHB = T // TCH            # token chunks per batch
    TB = T // 128             # 128-token blocks per batch
    SCALE = float(HD) ** -0.5
    Exp = mybir.ActivationFunctionType.Exp

    assert TQ == TCH, "A2A split layout assumes T == 2048"
    nc = bacc.Bacc("TRN2", target_bir_lowering=False, debug=False,
                   num_devices=NCORES)

    xT_d = nc.dram_tensor("xT", [D, TOK], bf16, kind="ExternalInput")
    wqkT_d = nc.dram_tensor("wqkT", [D, 6 * HD], bf16, kind="ExternalInput")
    woutT_d = nc.dram_tensor("woutT", [D, D], bf16, kind="ExternalInput")
    cosT_d = nc.dram_tensor("cosT", [HD, T], bf16, kind="ExternalInput")
    sinS_d = nc.dram_tensor("sinS", [HD, T], bf16, kind="ExternalInput")
    out_d = nc.dram_tensor("out", [THALF, D], f32, kind="ExternalOutput")

    xT_v = xT_d.ap().rearrange("(cb p) t -> p cb t", p=128)
    wqkT_v = wqkT_d.ap().rearrange("(cb p) f -> p cb f", p=128)
    woutT_v = woutT_d.ap().rearrange("(cb p) o -> p cb o", p=128)

    with tile.TileContext(nc) as tc:
        with (
            tc.tile_pool(name="const", bufs=1) as constp,
            tc.tile_pool(name="dram", bufs=1, space="DRAM") as dramp,
        ):
            cos_sb = constp.tile([128, T], bf16, name="cos_sb")
            sin_sb = constp.tile([128, T], bf16, name="sin_sb")
            mask_sb = constp.tile([128, 4, TCH], bf16, name="mask_sb")
            ones_sb = constp.tile([128, 128], bf16, name="ones_sb")
            nc.gpsimd.memset(mask_sb[:], 1.0)
            for jd in range(4):
                # keep 1.0 where  tq_rel - tk_rel - 128*jd >= 0  else 0
                nc.gpsimd.affine_select(
                    out=mask_sb[:, jd, :], in_=mask_sb[:, jd, :],
                    compare_op=mybir.AluOpType.is_ge, fill=0.0,
                    base=-128 * jd, pattern=[[1, TCH]], channel_multiplier=-1,
                )
            nc.gpsimd.memset(ones_sb[:], 1.0)

            # per (tq-half, local head) AllToAll bounce buffers
            a2a_in = [[dramp.tile([NCORES, 128, TQ], bf16,
                                  name=f"a2a_in{p}{h}") for h in range(HL)]
                      for p in range(2)]
            a2a_out = [[dramp.tile([NCORES, 128, TQ], bf16,
                                   name=f"a2a_out{p}{h}") for h in range(HL)]
                       for p in range(2)]

            with tc.tile_pool(name="qkv", bufs=1) as qkvp:
                qT = [[qkvp.tile([128, T], bf16, name=f"qT_{hl}_{b}")
                       for b in range(B)] for hl in range(HL)]
                kT = [[qkvp.tile([128, T], bf16, name=f"kT_{hl}_{b}")
                       for b in range(B)] for hl in range(HL)]
                V = [qkvp.tile([128, TB, 2 * HD], bf16, name=f"V_{b}")
                     for b in range(B)]

                # -------- Phase 1: QKV projection + RoPE ------------------
                with (
                    tc.tile_pool(name="wqk", bufs=1) as wqkp,
                    tc.tile_pool(name="xin", bufs=2) as xp,
                    tc.tile_pool(name="ps_qk", bufs=3, space="PSUM") as psqk,
                    tc.tile_pool(name="ps_v", bufs=2, space="PSUM") as psv,
                    tc.tile_pool(name="rope", bufs=3) as ropep,
                ):
                    wqk_sb = wqkp.tile([128, CB, 6 * HD], bf16,
                                       name="wqk_sb")
                    nc.sync.dma_start(wqk_sb[:, 0:CB // 2, :],
                                      wqkT_v[:, 0:CB // 2, :])
                    nc.sync.dma_start(wqk_sb[:, CB // 2:CB, :],
                                      wqkT_v[:, CB // 2:CB, :])

                    for ch in range(NCH):
                        b, cc = divmod(ch, CHB)
                        t0 = cc * TCH
                        xpan = xp.tile([128, CB, TCH], bf16, tag="xpan",
                                       name=f"xpan_{ch}")
                        # first panel rides the idle ACT HWDGE queue so it
                        # overlaps the weight load on the Sync queue
                        eng = nc.scalar if ch == 0 else nc.sync
                        for g in range(2):
                            eng.dma_start(
                                xpan[:, g * CB // 2:(g + 1) * CB // 2, :],
                                xT_v[:, g * CB // 2:(g + 1) * CB // 2,
                                     ch * TCH:(ch + 1) * TCH])
                        if ch == 0:
                            # behind the critical first weight/x loads
                            nc.sync.dma_start(cos_sb[:], cosT_d[:, :])
                            nc.sync.dma_start(sin_sb[:], sinS_d[:, :])

                        for f in range(4):  # q_h0 q_h1 k_h0 k_h1
                            ps = psqk.tile([128, TCH], f32, tag="qk",
                                           name=f"psqk_{ch}_{f}")
                            for cb in range(CB):
                                nc.tensor.matmul(
                                    ps[:],
                                    lhsT=wqk_sb[:, cb, f * 128:(f + 1) * 128],
                                    rhs=xpan[:, cb, :],
                                    start=(cb == 0), stop=(cb == CB - 1))
                            qraw = ropep.tile([128, TCH], bf16, tag="qraw",
                                              name=f"qraw_{ch}_{f}")
                            nc.scalar.copy(qraw[:], ps[:])
                            rot = ropep.tile([128, TCH], bf16, tag="rot",
                                             name=f"rot_{ch}_{f}")
                            nc.vector.stream_shuffle(rot[:], qraw[:],
                                                     mask=_SHUF)
                            t1 = ropep.tile([128, TCH], bf16, tag="t1",
                                            name=f"t1_{ch}_{f}")
                            nc.vector.tensor_mul(
                                t1[:], qraw[:], cos_sb[:, t0:t0 + TCH])
                            nc.vector.tensor_mul(
                                rot[:], rot[:], sin_sb[:, t0:t0 + TCH])
                            dest = (qT if f < 2 else kT)[f % 2][b]
                            nc.vector.tensor_add(
                                dest[:, t0:t0 + TCH], t1[:], rot[:])

                        for tb in range(TCH // 128):  # v
                            pv = psv.tile([128, 2 * HD], f32, tag="v",
                                          name=f"psv_{ch}_{tb}")
                            for cb in range(CB):
                                nc.tensor.matmul(
                                    pv[:],
                                    lhsT=xpan[:, cb, tb * 128:(tb + 1) * 128],
                                    rhs=wqk_sb[:, cb, 4 * 128:6 * 128],
                                    start=(cb == 0), stop=(cb == CB - 1))
                            nc.scalar.copy(V[b][:, cc * 4 + tb, :], pv[:])

                # -------- Phase 2: attention + resharding -----------------
                attnall_t = []
                wout_pre = {}
                with (
                    tc.tile_pool(name="attn", bufs=2) as attnp,
                    tc.tile_pool(name="wout", bufs=2) as woutp,
                ):
                  with (
                    tc.tile_pool(name="ps_st", bufs=2, space="PSUM") as psst,
                    tc.tile_pool(name="ps_acc", bufs=4, space="PSUM") as psacc,
                    tc.tile_pool(name="pexp", bufs=3) as pexpp,
                    tc.tile_pool(name="onorm", bufs=3) as onp,
                  ):
                    for part in range(2):
                        # layout [128, i(core), hl, t] == attnallT c order
                        attnall = attnp.tile([128, CB // 2, HL, TQ], bf16,
                                             tag="attnall",
                                             name=f"attnall_{part}")
                        attnall_t.append(attnall)
                        for hl in range(HL):
                            for tqc in range(part, CHB, 2):
                                for b in range(B):
                                    _attn_chunk(
                                        nc, mybir, psst, psacc, pexpp, onp,
                                        qT, kT, V, mask_sb, ones_sb,
                                        a2a_in[part][hl], b, hl, tqc,
                                        SCALE, Exp, f32, bf16)
                            # reshard this (half, head) while the rest of
                            # attention / the output projection runs
                            nc.gpsimd.collective_compute(
                                "AllToAll", mybir.AluOpType.bypass,
                                replica_groups=[list(range(NCORES))],
                                ins=[a2a_in[part][hl].opt()],
                                outs=[a2a_out[part][hl].opt()],
                            )
                            # critical post-collective load on the (idle)
                            # gpsimd queue, not stuck behind Sync DMAs
                            nc.gpsimd.dma_start(
                                attnall[:, :, hl, :],
                                a2a_out[part][hl].rearrange(
                                    "i p t -> p i t"))
                            if part == 0 and hl == 1:
                                for oc in range(2):
                                    w = woutp.tile(
                                        [128, CB, TCH], bf16, tag="wout",
                                        name=f"wout_0_{oc}")
                                    nc.gpsimd.dma_start(
                                        w[:],
                                        woutT_v[:, :,
                                                oc * TCH:(oc + 1) * TCH])
                                    wout_pre[(0, oc)] = w

                  # -------- Phase 3: output projection --------------------
                  with (
                    tc.tile_pool(name="ps_out", bufs=2, space="PSUM") as pso,
                    tc.tile_pool(name="o3", bufs=3) as o3p,
                  ):
                    last_mm = None
                    first_mm_p1 = None
                    for part in range(2):
                        attnall = attnall_t[part]
                        for oc in range(4):
                            if (part, oc) in wout_pre:
                                w = wout_pre[(part, oc)]
                            else:
                                w = woutp.tile([128, CB, TCH], bf16,
                                               tag="wout",
                                               name=f"wout_{part}_{oc}")
                                nc.sync.dma_start(
                                    w[:],
                                    woutT_v[:, :, oc * TCH:(oc + 1) * TCH])
                            for tb in range(TQ // 128):
                                po = pso.tile([128, TCH], f32, tag="out",
                                              name=f"po_{part}_{oc}_{tb}")
                                for cb in range(CB):
                                    mm = nc.tensor.matmul(
                                        po[:],
                                        lhsT=attnall[:, cb // 2, cb % 2,
                                                     tb * 128:(tb + 1) * 128],
                                        rhs=w[:, cb, :],
                                        start=(cb == 0),
                                        stop=(cb == CB - 1))
                                    if part == 1 and first_mm_p1 is None:
                                        first_mm_p1 = mm
                                    if part == 0:
                                        last_mm = mm
                                ot = o3p.tile([128, TCH], f32, tag="o3",
                                              name=f"ot_{part}_{oc}_{tb}")
                                nc.scalar.copy(ot[:], po[:])
                                nc.sync.dma_start(
                                    out_d[part * TQ + tb * 128:
                                          part * TQ + (tb + 1) * 128,
                                          oc * TCH:(oc + 1) * TCH],
                                    ot[:])
                    # keep the two out-proj halves in emission order on the
                    # PE so part 1 (gated on the later collectives) cannot
                    # starve part 0's remaining matmuls
                    tile.add_dep_helper(
                        first_mm_p1.ins, last_mm.ins, sync=False,
                        reason="outproj part order")

    nc.compile()
    return nc


def _attn_chunk(nc, mybir, psst, psacc, pexpp, onp, qT, kT, V, mask_sb,
                ones_sb, a2a_in_ph, b, hl, tqc, SCALE, Exp, f32, bf16):
    """One (batch, head, 512-query-chunk) of causal attention."""
    ntk = (tqc + 1) * (TCH // 128)
    npair = ntk // 2
    q_sl = qT[hl][b][:, tqc * TCH:(tqc + 1) * TCH]
    av = psacc.tile([128, TCH], f32, tag="acc", name=f"av_{b}_{hl}_{tqc}")
    ones_ps = psacc.tile([128, TCH], f32, tag="acc",
                         name=f"on_{b}_{hl}_{tqc}")
    pexp_t = {}

    def emit_pair(p):
        st = psst.tile([128, 2 * TCH], f32, tag="st",
                       name=f"st_{b}_{hl}_{tqc}_{p}")
        for jj in range(2):
            j = 2 * p + jj
            nc.tensor.matmul(
                st[:, jj * TCH:(jj + 1) * TCH],
                lhsT=kT[hl][b][:, j * 128:(j + 1) * 128],
                rhs=q_sl, start=True, stop=True)
        pe = pexpp.tile([128, 2 * TCH], bf16, tag="pexp",
                        name=f"pe_{b}_{hl}_{tqc}_{p}")
        nc.scalar.activation(pe[:], st[:], Exp, scale=SCALE)
        for jj in range(2):
            j = 2 * p + jj
            jd = j - (TCH // 128) * tqc
            if jd >= 0:  # diagonal block: causal mask
                sl = pe[:, jj * TCH:(jj + 1) * TCH]
                nc.vector.tensor_mul(sl, sl, mask_sb[:, jd, :])
        pexp_t[p] = pe

    emit_pair(0)
    for p in range(npair):
        if p + 1 < npair:
            emit_pair(p + 1)
        pe = pexp_t.pop(p)
        for jj in range(2):
            j = 2 * p + jj
            sl = pe[:, jj * TCH:(jj + 1) * TCH]
            first = j == 0
            last = j == ntk - 1
            nc.tensor.matmul(ones_ps[:], lhsT=ones_sb[:], rhs=sl,
                             start=first, stop=last, skip_group_check=True)
            nc.tensor.matmul(
                av[:], lhsT=V[b][:, j, hl * 128:(hl + 1) * 128], rhs=sl,
                start=first, stop=last, skip_group_check=True)

    recip = onp.tile([128, TCH], f32, tag="recip", name=f"rc_{b}_{hl}_{tqc}")
    nc.vector.reciprocal_approx_fast(recip[:], ones_ps[:])
    oT = onp.tile([128, TCH], bf16, tag="oT", name=f"oT_{b}_{hl}_{tqc}")
    nc.vector.tensor_mul(oT[:], av[:], recip[:])
    dj = b * 2 + tqc // 2
    nc.sync.dma_start(a2a_in_ph[dj, :, :], oT[:])


def prep_inputs(x, cos, sin, w_qkv, w_out, T=2048):
    """Host-side shard/layout prep. Returns in_maps for the 8 cores."""
    TOK = B * T
    xT = np.ascontiguousarray(x.reshape(TOK, D).T).astype(BF16)
    cosT = np.ascontiguousarray(cos.T[_PERM, :]).astype(BF16)
    sinS = np.ascontiguousarray(sin.T[_PERM, :] * _SIGN[:, None]).astype(BF16)
    woutT = np.ascontiguousarray(w_out.T).astype(BF16)
    in_maps = []
    for c in range(NCORES):
        rows = []
        for sec in range(2):  # q, k (perm'd)
            for hl in range(HL):
                h = 2 * c + hl
                w = w_qkv[sec * D + h * HD:sec * D + (h + 1) * HD, :]
                rows.append(w[_PERM, :])
        for hl in range(HL):  # v natural
            h = 2 * c + hl
            rows.append(w_qkv[2 * D + h * HD:2 * D + (h + 1) * HD, :])
        wqkT = np.ascontiguousarray(np.concatenate(rows, 0).T).astype(BF16)
        in_maps.append({"xT": xT, "wqkT": wqkT, "woutT": woutT,
                        "cosT": cosT, "sinS": sinS})
    return in_maps


_NC_CACHE = {}


def _get_nc(T=2048):
    if T not in _NC_CACHE:
        _NC_CACHE[T] = build_nc(T)
    return _NC_CACHE[T]


def kernel(x, cos, sin, w_qkv, w_out):
    import concourse.bass_utils as bass_utils

    T = x.shape[1]
    x = np.asarray(x, np.float32)
    cos = np.asarray(cos, np.float32)
    sin = np.asarray(sin, np.float32)
    w_qkv = np.asarray(w_qkv, np.float32)
    w_out = np.asarray(w_out, np.float32)

    nc = _get_nc(T)
    in_maps = prep_inputs(x, cos, sin, w_qkv, w_out, T)
    res = bass_utils.run_bass_kernel_spmd(nc, in_maps,
                                          core_ids=list(range(NCORES)))
    THALF = T // 2
    full = np.empty((B, T, D), np.float32)
    for j in range(NCORES):
        b, hf = divmod(j, 2)
        full[b, hf * THALF:(hf + 1) * THALF, :] = res.results[j]["out"]
    return full


# revision 19
# speedup vs baseline: 1.3360x; 1.0207x over previous
"""Causal self-attention (B=4, T=2048, D=2048, H=16, HD=128) on 8 Trainium2
NeuronCores.

Sharding: Megatron-style tensor parallel over heads for QKV projection +
attention (2 heads per core), then on-device AllToAlls reshard from
head-parallel to token-parallel (core j owns tokens of batch j//2, half j%2)
for the output projection.  Host only slices/transposes weights, replicates
activations, and concatenates the 8 output shards.

Device layouts (all matmul operands bf16, fp32 PSUM accumulation):
  xT    [D, B*T]   x transposed (contraction dim on partitions)
  qT/kT [128, T]   per (local head, batch); d-order permuted so the RoPE
                   rotate-half partner sits 16 partitions away (within a
                   32-partition quadrant, reachable by DVE stream_shuffle).
                   Any consistent permutation of d leaves q.k unchanged.
  V     [T, 128]   natural d order (feeds AV matmul lhsT and out-proj order)
  S^T   [tk, tq]   scores transposed: the softmax sum over the partition dim
                   is a ones-matmul on the PE (output rows are the broadcast
                   sums for free); no max-subtraction needed (logits ~
                   N(0,1), bounded ~ +-6, exp can't overflow).

The attention loop runs tq-half 0 (even 512-token chunks) then half 1, with
one AllToAll per (half, head) issued as soon as that head's chunks finish —
all four collectives overlap the remaining attention / output projection.
"""

import sys

for _p in ("/opt/trn_rl_repo", "/root/.axon_site/_ro/trn_rl_repo"):
    if _p not in sys.path:
        sys.path.insert(0, _p)

import numpy as np
import ml_dtypes

BF16 = ml_dtypes.bfloat16

B = 4
D = 2048
H = 16
HD = 128
NCORES = 8
HL = 2           # heads per core
CB = D // 128    # contraction blocks
TCH = 512        # token chunk (matmul moving free dim)


def _perm128():
    """Partition order for q/k head dims: quadrant g holds dims
    [16g,16g+16) (lo) then [64+16g, 64+16g+16) (hi), so the rotate-half
    partner of partition p is p+-16 (same 32-partition quadrant)."""
    perm = np.zeros(128, np.int64)
    for p in range(128):
        g, i = divmod(p, 32)
        perm[p] = g * 16 + i if i < 16 else 64 + g * 16 + (i - 16)
    return perm


_PERM = _perm128()
_SHUF = [(i + 16) % 32 for i in range(32)]  # out[i] = in[(i+16)%32]
_SIGN = np.where(np.arange(128) % 32 < 16, -1.0, 1.0).astype(np.float32)


def build_nc(T=2048):
    import concourse.bacc as bacc
    import concourse.tile as tile
    import concourse.mybir as mybir

    f32 = mybir.dt.float32
    bf16 = mybir.dt.bfloat16
    TOK = B * T
    THALF = T // 2
    TQ = THALF // 2           # tokens per (core, a2a part)
    NCH = TOK // TCH          # token chunks total
    CHB = T // TCH            # token chunks per batch
    TB = T // 128             # 128-token blocks per batch
    SCALE = float(HD) ** -0.5
    Exp = mybir.ActivationFunctionType.Exp

    assert TQ == TCH, "A2A split layout assumes T == 2048"
    nc = bacc.Bacc("TRN2", target_bir_lowering=False, debug=False,
                   num_devices=NCORES)

    xT_d = nc.dram_tensor("xT", [D, TOK], bf16, kind="ExternalInput")
    wqkT_d = nc.dram_tensor("wqkT", [D, 6 * HD], bf16, kind="ExternalInput")
    woutT_d = nc.dram_tensor("woutT", [D, D], bf16, kind="ExternalInput")
    cosT_d = nc.dram_tensor("cosT", [HD, T], bf16, kind="ExternalInput")
    sinS_d = nc.dram_tensor("sinS", [HD, T], bf16, kind="ExternalInput")
    out_d = nc.dram_tensor("out", [THALF, D], f32, kind="ExternalOutput")

    xT_v = xT_d.ap().rearrange("(cb p) t -> p cb t", p=128)
    wqkT_v = wqkT_d.ap().rearrange("(cb p) f -> p cb f", p=128)
    woutT_v = woutT_d.ap().rearrange("(cb p) o -> p cb o", p=128)

    with tile.TileContext(nc) as tc:
        with (
            tc.tile_pool(name="const", bufs=1) as constp,
            tc.tile_pool(name="dram", bufs=1, space="DRAM") as dramp,
        ):
            cos_sb = constp.tile([128, T], bf16, name="cos_sb")
            sin_sb = constp.tile([128, T], bf16, name="sin_sb")
            mask_sb = constp.tile([128, 4, TCH], bf16, name="mask_sb")
            ones_sb = constp.tile([128, 128], bf16, name="ones_sb")
            nc.gpsimd.memset(mask_sb[:], 1.0)
            for jd in range(4):
                # keep 1.0 where  tq_rel - tk_rel - 128*jd >= 0  else 0
                nc.gpsimd.affine_select(
                    out=mask_sb[:, jd, :], in_=mask_sb[:, jd, :],
                    compare_op=mybir.AluOpType.is_ge, fill=0.0,
                    base=-128 * jd, pattern=[[1, TCH]], channel_multiplier=-1,
                )
            nc.gpsimd.memset(ones_sb[:], 1.0)

            # per (tq-half, local head) AllToAll bounce buffers
            a2a_in = [[dramp.tile([NCORES, 128, TQ], bf16,
                                  name=f"a2a_in{p}{h}") for h in range(HL)]
                      for p in range(2)]
            a2a_out = [[dramp.tile([NCORES, 128, TQ], bf16,
                                   name=f"a2a_out{p}{h}") for h in range(HL)]
                       for p in range(2)]

            with tc.tile_pool(name="qkv", bufs=1) as qkvp:
                qT = [[qkvp.tile([128, T], bf16, name=f"qT_{hl}_{b}")
                       for b in range(B)] for hl in range(HL)]
                kT = [[qkvp.tile([128, T], bf16, name=f"kT_{hl}_{b}")
                       for b in range(B)] for hl in range(HL)]
                V = [qkvp.tile([128, TB, 2 * HD], bf16, name=f"V_{b}")
                     for b in range(B)]

                # -------- Phase 1: QKV projection + RoPE ------------------
                with (
                    tc.tile_pool(name="wqk", bufs=1) as wqkp,
                    tc.tile_pool(name="xin", bufs=2) as xp,
                    tc.tile_pool(name="ps_qk", bufs=3, space="PSUM") as psqk,
                    tc.tile_pool(name="ps_v", bufs=2, space="PSUM") as psv,
                    tc.tile_pool(name="rope", bufs=3) as ropep,
                ):
                    wqk_sb = wqkp.tile([128, CB, 6 * HD], bf16,
                                       name="wqk_sb")
                    nc.sync.dma_start(wqk_sb[:, 0:CB // 2, :],
                                      wqkT_v[:, 0:CB // 2, :])
                    nc.sync.dma_start(wqk_sb[:, CB // 2:CB, :],
                                      wqkT_v[:, CB // 2:CB, :])

                    for ch in range(NCH):
                        b, cc = divmod(ch, CHB)
                        t0 = cc * TCH
                        xpan = xp.tile([128, CB, TCH], bf16, tag="xpan",
                                       name=f"xpan_{ch}")
                        # first panel rides the idle ACT HWDGE queue so it
                        # overlaps the weight load on the Sync queue
                        eng = nc.scalar if ch == 0 else nc.sync
                        for g in range(2):
                            eng.dma_start(
                                xpan[:, g * CB // 2:(g + 1) * CB // 2, :],
                                xT_v[:, g * CB // 2:(g + 1) * CB // 2,
                                     ch * TCH:(ch + 1) * TCH])
                        if ch == 0:
                            # behind the critical first weight/x loads
                            nc.sync.dma_start(cos_sb[:], cosT_d[:, :])
                            nc.sync.dma_start(sin_sb[:], sinS_d[:, :])

                        for f in range(4):  # q_h0 q_h1 k_h0 k_h1
                            ps = psqk.tile([128, TCH], f32, tag="qk",
                                           name=f"psqk_{ch}_{f}")
                            for cb in range(CB):
                                nc.tensor.matmul(
                                    ps[:],
                                    lhsT=wqk_sb[:, cb, f * 128:(f + 1) * 128],
                                    rhs=xpan[:, cb, :],
                                    start=(cb == 0), stop=(cb == CB - 1))
                            qraw = ropep.tile([128, TCH], bf16, tag="qraw",
                                              name=f"qraw_{ch}_{f}")
                            nc.scalar.copy(qraw[:], ps[:])
                            rot = ropep.tile([128, TCH], bf16, tag="rot",
                                             name=f"rot_{ch}_{f}")
                            nc.vector.stream_shuffle(rot[:], qraw[:],
                                                     mask=_SHUF)
                            t1 = ropep.tile([128, TCH], bf16, tag="t1",
                                            name=f"t1_{ch}_{f}")
                            nc.vector.tensor_mul(
                                t1[:], qraw[:], cos_sb[:, t0:t0 + TCH])
                            nc.vector.tensor_mul(
                                rot[:], rot[:], sin_sb[:, t0:t0 + TCH])
                            dest = (qT if f < 2 else kT)[f % 2][b]
                            nc.vector.tensor_add(
                                dest[:, t0:t0 + TCH], t1[:], rot[:])

                        for tb in range(TCH // 128):  # v
                            pv = psv.tile([128, 2 * HD], f32, tag="v",
                                          name=f"psv_{ch}_{tb}")
                            for cb in range(CB):
                                nc.tensor.matmul(
                                    pv[:],
                                    lhsT=xpan[:, cb, tb * 128:(tb + 1) * 128],
                                    rhs=wqk_sb[:, cb, 4 * 128:6 * 128],
                                    start=(cb == 0), stop=(cb == CB - 1))
                            nc.scalar.copy(V[b][:, cc * 4 + tb, :], pv[:])

                # -------- Phase 2: attention + resharding -----------------
                attnall_t = []
                wout_pre = {}
                with (
                    tc.tile_pool(name="attn", bufs=2) as attnp,
                    tc.tile_pool(name="wout", bufs=2) as woutp,
                ):
                  with (
                    tc.tile_pool(name="ps_st", bufs=2, space="PSUM") as psst,
                    tc.tile_pool(name="ps_acc", bufs=4, space="PSUM") as psacc,
                    tc.tile_pool(name="pexp", bufs=4) as pexpp,
                    tc.tile_pool(name="onorm", bufs=3) as onp,
                  ):
                    for part in range(2):
                        # layout [128, i(core), hl, t] == attnallT c order
                        attnall = attnp.tile([128, CB // 2, HL, TQ], bf16,
                                             tag="attnall",
                                             name=f"attnall_{part}")
                        attnall_t.append(attnall)
                        for hl in range(HL):
                            for tqc in range(part, CHB, 2):
                                for b in range(B):
                                    _attn_chunk(
                                        nc, mybir, psst, psacc, pexpp, onp,
                                        qT, kT, V, mask_sb, ones_sb,
                                        a2a_in[part][hl], b, hl, tqc,
                                        SCALE, Exp, f32, bf16)
                            # reshard this (half, head) while the rest of
                            # attention / the output projection runs
                            nc.gpsimd.collective_compute(
                                "AllToAll", mybir.AluOpType.bypass,
                                replica_groups=[list(range(NCORES))],
                                ins=[a2a_in[part][hl].opt()],
                                outs=[a2a_out[part][hl].opt()],
                            )
                            # critical post-collective load on the (idle)
                            # gpsimd queue, not stuck behind Sync DMAs
                            nc.gpsimd.dma_start(
                                attnall[:, :, hl, :],
                                a2a_out[part][hl].rearrange(
                                    "i p t -> p i t"))
                            if part == 0 and hl == 1:
                                for oc in range(2):
                                    w = woutp.tile(
                                        [128, CB, TCH], bf16, tag="wout",
                                        name=f"wout_0_{oc}")
                                    nc.gpsimd.dma_start(
                                        w[:],
                                        woutT_v[:, :,
                                                oc * TCH:(oc + 1) * TCH])
                                    wout_pre[(0, oc)] = w

                  # -------- Phase 3: output projection --------------------
                  with (
                    tc.tile_pool(name="ps_out", bufs=2, space="PSUM") as pso,
                    tc.tile_pool(name="o3", bufs=3) as o3p,
                  ):
                    last_mm = None
                    first_mm_p1 = None
                    for part in range(2):
                        attnall = attnall_t[part]
                        for oc in range(4):
                            if (part, oc) in wout_pre:
                                w = wout_pre[(part, oc)]
                            else:
                                w = woutp.tile([128, CB, TCH], bf16,
                                               tag="wout",
                                               name=f"wout_{part}_{oc}")
                                nc.sync.dma_start(
                                    w[:],
                                    woutT_v[:, :, oc * TCH:(oc + 1) * TCH])
                            for tb in range(TQ // 128):
                                po = pso.tile([128, TCH], f32, tag="out",
                                              name=f"po_{part}_{oc}_{tb}")
                                for cb in range(CB):
                                    mm = nc.tensor.matmul(
                                        po[:],
                                        lhsT=attnall[:, cb // 2, cb % 2,
                                                     tb * 128:(tb + 1) * 128],
                                        rhs=w[:, cb, :],
                                        start=(cb == 0),
                                        stop=(cb == CB - 1))
                                    if part == 1 and first_mm_p1 is None:
                                        first_mm_p1 = mm
                                    if part == 0:
                                        last_mm = mm
                                ot = o3p.tile([128, TCH], f32, tag="o3",
                                              name=f"ot_{part}_{oc}_{tb}")
                                nc.scalar.copy(ot[:], po[:])
                                nc.sync.dma_start(
                                    out_d[part * TQ + tb * 128:
                                          part * TQ + (tb + 1) * 128,
                                          oc * TCH:(oc + 1) * TCH],
                                    ot[:])
                    # keep the two out-proj halves in emission order on the
                    # PE so part 1 (gated on the later collectives) cannot
                    # starve part 0's remaining matmuls
                    tile.add_dep_helper(
                        first_mm_p1.ins, last_mm.ins, sync=False,
                        reason="outproj part order")

    nc.compile()
    return nc


def _attn_chunk(nc, mybir, psst, psacc, pexpp, onp, qT, kT, V, mask_sb,
                ones_sb, a2a_in_ph, b, hl, tqc, SCALE, Exp, f32, bf16):
    """One (batch, head, 512-query-chunk) of causal attention."""
    ntk = (tqc + 1) * (TCH // 128)
    npair = ntk // 2
    q_sl = qT[hl][b][:, tqc * TCH:(tqc + 1) * TCH]
    av = psacc.tile([128, TCH], f32, tag="acc", name=f"av_{b}_{hl}_{tqc}")
    ones_ps = psacc.tile([128, TCH], f32, tag="acc",
                         name=f"on_{b}_{hl}_{tqc}")
    pexp_t = {}

    def col0(j):
        """First valid tq column for tk-block j (causal: tq >= tk)."""
        jd = j - (TCH // 128) * tqc
        return 128 * jd if jd > 0 else 0

    def emit_pair(p):
        st = psst.tile([128, 2 * TCH], f32, tag="st",
                       name=f"st_{b}_{hl}_{tqc}_{p}")
        pe = pexpp.tile([128, 2 * TCH], bf16, tag="pexp",
                        name=f"pe_{b}_{hl}_{tqc}_{p}")
        for jj in range(2):
            j = 2 * p + jj
            c0 = col0(j)
            nc.tensor.matmul(
                st[:, jj * TCH + c0:(jj + 1) * TCH],
                lhsT=kT[hl][b][:, j * 128:(j + 1) * 128],
                rhs=q_sl[:, c0:TCH], start=True, stop=True)
        if col0(2 * p) == 0 and col0(2 * p + 1) == 0:
            nc.scalar.activation(pe[:], st[:], Exp, scale=SCALE)
        else:
            for jj in range(2):
                c0 = col0(2 * p + jj)
                nc.scalar.activation(
                    pe[:, jj * TCH + c0:(jj + 1) * TCH],
                    st[:, jj * TCH + c0:(jj + 1) * TCH], Exp, scale=SCALE)
        for jj in range(2):
            j = 2 * p + jj
            jd = j - (TCH // 128) * tqc
            if jd >= 0:  # diagonal block: causal mask on its triangle
                c0 = col0(j)
                sl = pe[:, jj * TCH + c0:(jj + 1) * TCH]
                nc.vector.tensor_mul(sl, sl, mask_sb[:, jd, c0:TCH])
        pexp_t[p] = pe

    emit_pair(0)
    for p in range(npair):
        if p + 1 < npair:
            emit_pair(p + 1)
        pe = pexp_t.pop(p)
        for jj in range(2):
            j = 2 * p + jj
            c0 = col0(j)
            sl = pe[:, jj * TCH + c0:(jj + 1) * TCH]
            first = j == 0  # always full width: sets has_written everywhere
            last = j == ntk - 1
            nc.tensor.matmul(ones_ps[:, c0:TCH], lhsT=ones_sb[:], rhs=sl,
                             start=first, stop=last, skip_group_check=True)
            nc.tensor.matmul(
                av[:, c0:TCH],
                lhsT=V[b][:, j, hl * 128:(hl + 1) * 128], rhs=sl,
                start=first, stop=last, skip_group_check=True)

    recip = onp.tile([128, TCH], f32, tag="recip", name=f"rc_{b}_{hl}_{tqc}")
    nc.vector.reciprocal_approx_fast(recip[:], ones_ps[:])
    oT = onp.tile([128, TCH], bf16, tag="oT", name=f"oT_{b}_{hl}_{tqc}")
    nc.vector.tensor_mul(oT[:], av[:], recip[:])
    dj = b * 2 + tqc // 2
    nc.sync.dma_start(a2a_in_ph[dj, :, :], oT[:])


def prep_inputs(x, cos, sin, w_qkv, w_out, T=2048):
    """Host-side shard/layout prep. Returns in_maps for the 8 cores."""
    TOK = B * T
    xT = np.ascontiguousarray(x.reshape(TOK, D).T).astype(BF16)
    cosT = np.ascontiguousarray(cos.T[_PERM, :]).astype(BF16)
    sinS = np.ascontiguousarray(sin.T[_PERM, :] * _SIGN[:, None]).astype(BF16)
    woutT = np.ascontiguousarray(w_out.T).astype(BF16)
    in_maps = []
    for c in range(NCORES):
        rows = []
        for sec in range(2):  # q, k (perm'd)
            for hl in range(HL):
                h = 2 * c + hl
                w = w_qkv[sec * D + h * HD:sec * D + (h + 1) * HD, :]
                rows.append(w[_PERM, :])
        for hl in range(HL):  # v natural
            h = 2 * c + hl
            rows.append(w_qkv[2 * D + h * HD:2 * D + (h + 1) * HD, :])
        wqkT = np.ascontiguousarray(np.concatenate(rows, 0).T).astype(BF16)
        in_maps.append({"xT": xT, "wqkT": wqkT, "woutT": woutT,
                        "cosT": cosT, "sinS": sinS})
    return in_maps


_NC_CACHE = {}


def _get_nc(T=2048):
    if T not in _NC_CACHE:
        _NC_CACHE[T] = build_nc(T)
    return _NC_CACHE[T]


def kernel(x, cos, sin, w_qkv, w_out):
    import concourse.bass_utils as bass_utils

    T = x.shape[1]
    x = np.asarray(x, np.float32)
    cos = np.asarray(cos, np.float32)
    sin = np.asarray(sin, np.float32)
    w_qkv = np.asarray(w_qkv, np.float32)
    w_out = np.asarray(w_out, np.float32)

    nc = _get_nc(T)
    in_maps = prep_inputs(x, cos, sin, w_qkv, w_out, T)
    res = bass_utils.run_bass_kernel_spmd(nc, in_maps,
                                          core_ids=list(range(NCORES)))
    THALF = T // 2
    full = np.empty((B, T, D), np.float32)
    for j in range(NCORES):
        b, hf = divmod(j, 2)
        full[b, hf * THALF:(hf + 1) * THALF, :] = res.results[j]["out"]
    return full
